# revision 35
# baseline (speedup 1.0000x reference)
"""Trainium2 Bass kernel for MQA attention with RMSNorm + positional bias.

Reference computation (per core, seq-sharded over 8 cores):
  xn = rmsnorm(x) * gamma
  q = (xn @ wq) * scale   (16 heads x 128)     k = xn @ wk    v = xn @ wv
  sim = q @ k^T + pos_bias ; masked (non-causal entries := 1e-10)
  attn = softmax(sim); out = (attn @ v, concat heads) @ wo

Sharding: core m owns query rows of global 128-row tiles {m, m+8}. K/V
(shared MQA head) are computed from own rows and AllGathered in ONE
combined collective (k fp32 cols [0:256], v fp32 cols [256:512]).

Design (v2 rewrite of the 421us baseline):
- pos_bias AND the causal mask are folded into one additive host tensor
  pb (bf16, -1e9 at masked cols incl. a packed diag slot). pb is
  accumulated into the sim PSUM by the PE itself (identity-weight bf16
  matmul, 1 cycle/row) - no elementwise mask/mult pass and no gated-k
  copies. Masked cols then vanish through exp naturally, and the row max
  (V) can read raw PSUM.
- psim chunks are exactly 512 wide (1 PSUM bank): [512,384+128diag] for
  t=0 (7 main tiles), [512,512,512,384+128diag] for t=1 (15 main tiles).
- softmax denominator comes free via exp's accum_out; the masked-value
  correction of the reference (exp(1e-10-m) per masked col) is
  cnt*em as in the baseline.
- Normalization is folded into the P^T transposes: rhs = diag(rec)
  instead of identity, so P^T arrives pre-normalized.
- attn@v runs transposed per 4-head group: out^T[d, 4*128 q] with v
  tiles as weights - output lands directly in oT layout (no per-unit
  normalize chain, no 32 output transposes), and the masked-value
  output correction em*rec*stail rides one extra matmul with
  rhs = blockdiag(c_h), c_h = em_h*rec_h.
- k/v projections accumulate chunk-by-chunk interleaved with the x^T
  transposes so the combined AllGather triggers ~20us in; qproj +
  wq streaming fill the mesh-latency window.
"""

import os

import numpy as np
import ml_dtypes

import concourse.bass as bass
import concourse.mybir as mybir
import concourse.tile as tile
from concourse import bacc, masks
from concourse.bass_utils import run_bass_kernel_spmd

SEQ = 2048
DIM = 2048
H = 16
DH = 128
P = 128
N_CORES = 8
MQ = SEQ // N_CORES      # 256 query rows per core
NQT = MQ // P            # 2 query tiles per core
CD = DIM // P            # 16 contraction chunks
NS = SEQ // P            # 16 seq tiles
NMT = {0: 7, 1: 15}      # main key tiles per unit type
NCH = {0: 2, 1: 4}       # 512-wide sim chunks per unit type (diag packed)
LW = {0: 1024, 1: 2048}  # pb / pexp width per unit type
NT = H * NQT             # 32 pipeline units
GH = 4                   # heads per attn@v group
WOPF = 8                 # wo heads prefetched during the pipeline
SCALE = DH ** -0.5
EPS = 1e-5

FP = mybir.dt.float32
F16 = mybir.dt.float16
FR = mybir.dt.float32r
BF = mybir.dt.bfloat16
AF = mybir.ActivationFunctionType
ALU = mybir.AluOpType
AX = mybir.AxisListType

last_exec_time_ns = None


def _rms_scale_rows(nc, pool, xt, tag):
    """In-place x *= rsqrt(mean(x^2)+eps) for a [P, DIM] tile."""
    sq = pool.tile([P, DIM], FP, tag="sq_scratch", name="sq_scratch", bufs=1)
    ssq = pool.tile([P, 1], FP, tag=f"ssq{tag}", name=f"ssq{tag}")
    nc.scalar.activation(sq[:], xt[:], AF.Square, accum_out=ssq[:])
    nc.vector.tensor_scalar(ssq[:], ssq[:], 1.0 / DIM, EPS, ALU.mult, ALU.add)
    nc.scalar.sqrt(ssq[:], ssq[:])
    nc.vector.reciprocal(ssq[:], ssq[:])
    nc.vector.tensor_scalar_mul(xt[:], xt[:], ssq[:])


def build():
    nc = bacc.Bacc("TRN2", target_bir_lowering=False, debug=False,
                   num_devices=N_CORES)
    xq_d = nc.dram_tensor("xq", [NQT * 8 * P, 256], FP,
                          kind="ExternalInput")
    pbA_d = nc.dram_tensor("pbA", [H * P, LW[0]], BF, kind="ExternalInput")
    pbB_d = nc.dram_tensor("pbB", [H * P, LW[1]], BF, kind="ExternalInput")
    lo_d = nc.dram_tensor("lones", [P, P], BF, kind="ExternalInput")
    cnt_d = nc.dram_tensor("cnt", [P, NQT], FP, kind="ExternalInput")
    hz_d = nc.dram_tensor("hz", [P, NQT * NS], BF, kind="ExternalInput")
    wq_d = nc.dram_tensor("wq", [P, H * CD * DH], FR, kind="ExternalInput")
    wk_d = nc.dram_tensor("wk", [P, CD * DH], FR, kind="ExternalInput")
    wv_d = nc.dram_tensor("wv", [P, CD * DH], FR, kind="ExternalInput")
    wo_d = nc.dram_tensor("wo", [H * DH, DIM], BF, kind="ExternalInput")
    out_d = nc.dram_tensor("out", [MQ, DIM], FP, kind="ExternalOutput")
    DBG = os.environ.get("KERNEL_DEBUG") == "1"
    if DBG:
        dbg_qT = nc.dram_tensor("dbg_qT", [P, H * MQ], FP,
                                kind="ExternalOutput")
        dbg_kT = nc.dram_tensor("dbg_kT", [P, SEQ], FP, kind="ExternalOutput")
        dbg_vsb = nc.dram_tensor("dbg_vsb", [P, NS * P], FP,
                                 kind="ExternalOutput")
        dbg_st = nc.dram_tensor("dbg_st", [P, NQT * DH], FP,
                                kind="ExternalOutput")
        dbg_oT = nc.dram_tensor("dbg_oT", [P, H * MQ], FP,
                                kind="ExternalOutput")
        dbg_px = nc.dram_tensor("dbg_px", [P, LW[0] + LW[1]], FP,
                                kind="ExternalOutput")
        dbg_sc = nc.dram_tensor("dbg_sc", [P, 8], FP, kind="ExternalOutput")

    with tile.TileContext(nc) as tc, \
         tc.tile_pool(name="singles", bufs=1) as singles:
        # ---- persistent tiles --------------------------------------------
        ident = singles.tile([P, P], FP, tag="ident", name="ident")
        masks.make_identity(nc, ident[:])
        identb = singles.tile([P, P], BF, tag="identb", name="identb")
        masks.make_identity(nc, identb[:])
        cnt = singles.tile([P, NQT], FP, tag="cnt", name="cnt")

        qT = singles.tile([P, H, MQ], FR, tag="qT", name="qT")
        kTA = singles.tile([P, LW[0]], FR, tag="kTA", name="kTA")
        kTB = singles.tile([P, LW[1]], FR, tag="kTB", name="kTB")
        vsb = singles.tile([P, NS, P], BF, tag="vsb", name="vsb")
        vown = singles.tile([P, NQT, P], BF, tag="vown", name="vown")
        stail = singles.tile([P, NQT, DH], BF, tag="stail", name="stail")
        oT = singles.tile([P, H, MQ], BF, tag="oT", name="oT")
        wo_sb = singles.tile([P, WOPF, DIM], BF, tag="wo_sb", name="wo_sb")
        hz_sb = singles.tile([P, NQT, NS], BF, tag="hz", name="hz_sb")
        lones = singles.tile([P, P], BF, tag="lones", name="lones")
        ones1 = singles.tile([1, P], BF, tag="ones1", name="ones1")
        vsuf = singles.tile([1, DH], BF, tag="vsuf", name="vsuf")

        with tc.tile_pool(name="xnTqp", bufs=1) as xnTqp, \
             tc.tile_pool(name="pos", bufs=1) as posp, \
             tc.tile_pool(name="wqp", bufs=3) as wqp:
            xnT = xnTqp.tile([P, CD, MQ], FR, tag="xnT", name="xnT")
            WQH = {}
            PB = {}

            def qproj_dma(h, eng):
                wqh = wqp.tile([P, CD, DH], FR, tag="wqh",
                               name="wqh", bufs=3)
                for cb in range(4):
                    eng.dma_start(
                        out=wqh[:, cb * 4:(cb + 1) * 4, :],
                        in_=wq_d[:, h * CD * DH + cb * 512:
                                 h * CD * DH + (cb + 1) * 512])
                WQH[h] = wqh

            def pb_prefetch(i, eng0, eng1):
                h, t = i // NQT, i % NQT
                pbsel = pbA_d if t == 0 else pbB_d
                pb_t = posp.tile([P, LW[t]], BF, tag=f"pb{t}",
                                 name="pb", bufs=3)
                half = LW[t] // 2
                eng0.dma_start(out=pb_t[:, 0:half],
                               in_=pbsel[h * P:(h + 1) * P, 0:half])
                eng1.dma_start(out=pb_t[:, half:],
                               in_=pbsel[h * P:(h + 1) * P, half:])
                PB[i] = pb_t

            # ---- phase 0+1: xn^T, k/v proj, combined AllGather -----------
            with tc.tile_pool(name="ph0", bufs=1) as ph0, \
                 tc.tile_pool(name="kvw", bufs=1) as kvwp, \
                 tc.tile_pool(name="dram", bufs=1, space="DRAM") as dramp, \
                 tc.tile_pool(name="pstr0", bufs=2, space="PSUM") as pstr0, \
                 tc.tile_pool(name="psk", bufs=1, space="PSUM") as psk, \
                 tc.tile_pool(name="psv", bufs=1, space="PSUM") as psv, \
                 tc.tile_pool(name="pstv", bufs=1, space="PSUM") as pstv:
                wk_sb = kvwp.tile([P, CD, DH], FR, tag="wk", name="wk_sb")
                wv_sb = kvwp.tile([P, CD, DH], FR, tag="wv", name="wv_sb")
                # x load first: 8 column chunks spread over issue queues
                xnq = []
                engs = [nc.sync, nc.scalar, nc.gpsimd]
                ei = 0
                for t in range(NQT):
                    xt = ph0.tile([P, DIM], FP, tag=f"xq{t}", name=f"xq{t}")
                    for ci in range(8):
                        blk = (t * 8 + ci) * P
                        engs[ei % 3].dma_start(
                            out=xt[:, ci * 256:(ci + 1) * 256],
                            in_=xq_d[blk:blk + P, :])
                        ei += 1
                    xnq.append(xt)
                for ci in range(4):
                    nc.scalar.dma_start(
                        out=wk_sb[:, ci * 4:(ci + 1) * 4, :],
                        in_=wk_d[:, ci * 512:(ci + 1) * 512])
                    nc.gpsimd.dma_start(
                        out=wv_sb[:, ci * 4:(ci + 1) * 4, :],
                        in_=wv_d[:, ci * 512:(ci + 1) * 512])
                nc.vector.memset(ones1[:], 1.0)
                nc.scalar.dma_start(out=lones[:], in_=lo_d[:])
                nc.scalar.dma_start(out=hz_sb[:], in_=hz_d[:])
                nc.sync.dma_start(out=cnt[:], in_=cnt_d[:])
                for t in range(NQT):
                    _rms_scale_rows(nc, ph0, xnq[t], f"q{t}")
                # transposes interleaved with k/v projection accumulation
                # (gamma is folded into wq/wk/wv host-side)
                pk = psk.tile([P, MQ], FP, tag="pk", name="pk")
                pv = psv.tile([P, MQ], FP, tag="pv", name="pv")
                # t0 transposes stream while t1's x still loads; k/v proj
                # accumulate full-width (fp32r needs free>=256 for 1c/row)
                for c in range(CD):
                    pt = pstr0.tile([P, P], FP, tag="trq", name="trq")
                    nc.tensor.transpose(pt[:], xnq[0][:, c * P:(c + 1) * P],
                                        ident[:])
                    if c % 2 == 0:
                        nc.vector.tensor_copy(xnT[:, c, 0:P], pt[:])
                    else:
                        nc.scalar.copy(xnT[:, c, 0:P], pt[:])
                for c in range(CD + 1):
                    if c < CD:
                        pt = pstr0.tile([P, P], FP, tag="trq", name="trq")
                        nc.tensor.transpose(
                            pt[:], xnq[1][:, c * P:(c + 1) * P], ident[:])
                        if c % 2 == 0:
                            nc.vector.tensor_copy(xnT[:, c, P:MQ], pt[:])
                        else:
                            nc.scalar.copy(xnT[:, c, P:MQ], pt[:])
                    if c > 0:
                        nc.tensor.matmul(pk[:], lhsT=wk_sb[:, c - 1, :],
                                         rhs=xnT[:, c - 1, :],
                                         start=(c == 1), stop=(c == CD))
                        nc.tensor.matmul(pv[:], lhsT=wv_sb[:, c - 1, :],
                                         rhs=xnT[:, c - 1, :],
                                         start=(c == 1), stop=(c == CD))
                # k bounce (fp32) and v bounce (bf16, = vown layout)
                kbsb = kvwp.tile([P, MQ], F16, tag="kbsb", name="kbsb")
                nc.scalar.copy(kbsb[:], pk[:])
                vTs = kvwp.tile([P, MQ], FP, tag="vTs", name="vTs")
                nc.vector.tensor_copy(vTs[:], pv[:])
                for t in range(NQT):
                    ptv = pstv.tile([P, P], FP, tag="vtr", name="vtr")
                    nc.tensor.transpose(ptv[:], vTs[:, t * P:(t + 1) * P],
                                        ident[:])
                    nc.vector.tensor_copy(vown[:, t, :], ptv[:])
                # fused k(fp16-bits)+v(bf16) bounce: ONE mesh
                kv_bounce = dramp.tile([P, 2 * MQ], BF, tag="kvb",
                                       name="kv_bounce")
                kv_ag = dramp.tile([N_CORES, P, 2 * MQ], BF, tag="kvag",
                                   name="kv_ag", addr_space="Shared")
                nc.scalar.dma_start(out=kv_bounce[:, 0:MQ],
                                    in_=kbsb[:].bitcast(BF))
                nc.scalar.dma_start(out=kv_bounce[:, MQ:2 * MQ],
                                    in_=vown[:, :, :])
                rg = [list(range(N_CORES))]
                nc.gpsimd.collective_compute(
                    "AllGather", ALU.bypass, replica_groups=rg,
                    ins=[kv_bounce[:].opt()], outs=[kv_ag[:, :, :].opt()])
                # k unpack into per-t layouts (wide permuted-AP DMAs on the
                # gpsimd queue, which is dead between the two collectives)
                kstgA = kvwp.tile([P, LW[0]], F16, tag="kstgA",
                                  name="kstgA")
                kstgB = kvwp.tile([P, LW[1]], F16, tag="kstgB",
                                  name="kstgB")
                nc.scalar.copy(kstgA[:, NMT[0] * P:], pk[:, 0:P])
                nc.scalar.copy(kstgB[:, NMT[1] * P:], pk[:, P:MQ])

                def unpack(dst, r0, r1, h0, cast=None):
                    in_ = kv_ag[r0:r1, :, h0 * P:(h0 + 1) * P] \
                        .transpose([1, 0, 2])
                    if cast is not None:
                        in_ = in_.bitcast(cast)
                    nc.gpsimd.dma_start(out=dst, in_=in_)
                unpack(kstgA[:, 0:3 * P], 0, 3, 0, F16)
                unpack(kstgA[:, 3 * P:5 * P], 3, 5, 0, F16)
                unpack(kstgA[:, 5 * P:7 * P], 5, 7, 0, F16)
                nc.vector.tensor_copy(kTA[:], kstgA[:])
                unpack(kstgB[:, 0:4 * P], 0, 4, 0, F16)
                unpack(kstgB[:, 4 * P:8 * P], 4, 8, 0, F16)
                unpack(kstgB[:, 8 * P:12 * P], 0, 4, 1, F16)
                unpack(kstgB[:, 12 * P:15 * P], 4, 7, 1, F16)
                nc.vector.tensor_copy(kTB[:], kstgB[:])
                # prologue prefetches at the tail of phase-1's scalar queue:
                # transfers start only once the kproj path stops needing HBM
                for h in range(12):
                    qproj_dma(h, nc.scalar)
                for i in range(6):
                    pb_prefetch(i, nc.scalar, nc.scalar)
                VCTX = unpack

            # ---- phase 2: q proj + attention pipeline --------------------
            with tc.tile_pool(name="pexpp", bufs=1) as pexpp, \
                 tc.tile_pool(name="pts", bufs=1) as ptsp, \
                 tc.tile_pool(name="st", bufs=1) as stp, \
                 tc.tile_pool(name="ps2", bufs=1, space="PSUM") as ps2:

                def qproj_mm(h):
                    wqh = WQH.pop(h)
                    pq = ps2.tile([P, MQ], FP, tag="psim", name="pq",
                                  bufs=5)
                    for c in range(CD):
                        nc.tensor.matmul(pq[:], lhsT=wqh[:, c, :],
                                         rhs=xnT[:, c, :],
                                         start=(c == 0), stop=(c == CD - 1))
                    nc.scalar.copy(qT[:, h, :], pq[:])

                S = {}    # per-unit pipeline state
                PTD = {}  # per-(group, t) P^T staging tiles
                DC = {}   # per-(group, t) blockdiag(em*rec) tiles

                def stage_mm(i):
                    h, t = i // NQT, i % NQT
                    nch = NCH[t]
                    pb_t = PB.pop(i)
                    kTt = kTA if t == 0 else kTB
                    qsl = qT[:, h, t * P:(t + 1) * P]
                    psims = []
                    for c in range(nch):
                        psim = ps2.tile([P, 512], FP, tag="psim",
                                        name="psim", bufs=5)
                        nc.tensor.matmul(psim[:], lhsT=qsl,
                                         rhs=kTt[:, c * 512:(c + 1) * 512],
                                         start=True, stop=False)
                        nc.tensor.matmul(psim[:], lhsT=identb[:],
                                         rhs=pb_t[:, c * 512:(c + 1) * 512],
                                         start=False, stop=True)
                        psims.append(psim)
                    mxc = stp.tile([P, 4], FP, tag="mxc", name="mxc",
                                   bufs=3)
                    for c in range(nch):
                        nc.vector.tensor_reduce(mxc[:, c:c + 1], psims[c][:],
                                                axis=AX.X, op=ALU.max)
                    negmax = stp.tile([P, 1], FP, tag="negmax",
                                      name="negmax", bufs=4)
                    nc.vector.tensor_reduce(negmax[:], mxc[:, 0:nch],
                                            axis=AX.X, op=ALU.max,
                                            negate=True)
                    nc.vector.tensor_scalar(negmax[:], negmax[:], 1.0, 0.0,
                                            ALU.mult, ALU.min)
                    S[i] = {"psims": psims, "negmax": negmax}

                def stage_exp(i):
                    h, t = i // NQT, i % NQT
                    g, hh = h // GH, h % GH
                    nch = NCH[t]
                    st = S[i]
                    negmax = st["negmax"]
                    pexp = pexpp.tile([P, LW[t]], BF, tag=f"pexp{t}",
                                      name="pexp", bufs=3)
                    sexp = stp.tile([P, 4], FP, tag="sexp", name="sexp",
                                    bufs=3)
                    for c in range(nch):
                        nc.scalar.activation(pexp[:, c * 512:(c + 1) * 512],
                                             st["psims"][c][:], AF.Exp,
                                             bias=negmax[:],
                                             accum_out=sexp[:, c:c + 1])
                    em = stp.tile([P, 1], FP, tag="em", name="em", bufs=4)
                    nc.scalar.activation(em[:], negmax[:], AF.Exp)
                    ssum = stp.tile([P, 1], FP, tag="ssum", name="ssum",
                                    bufs=4)
                    nc.vector.tensor_reduce(ssum[:], sexp[:, 0:nch],
                                            axis=AX.X, op=ALU.add)
                    ve = nc.gpsimd if 2 <= i < 20 else nc.vector
                    ssum2 = stp.tile([P, 1], FP, tag="ssum2", name="ssum2",
                                     bufs=4)
                    nc.vector.scalar_tensor_tensor(
                        ssum2[:], cnt[:, t:t + 1], em[:], ssum[:],
                        op0=ALU.mult, op1=ALU.add)
                    rec = stp.tile([P, 1], FP, tag="rec", name="rec", bufs=4)
                    nc.vector.reciprocal(rec[:], ssum2[:])
                    drec = stp.tile([P, P], BF, tag="drec", name="drec",
                                    bufs=4)
                    ve.tensor_scalar_mul(drec[:], identb[:], rec[:])
                    cc_ = stp.tile([P, 1], FP, tag="cc", name="cc", bufs=4)
                    ve.tensor_tensor(cc_[:], em[:], rec[:], op=ALU.mult)
                    if (g, t) not in DC:
                        DC[(g, t)] = stp.tile([P, GH * P], BF, tag=f"dc{t}",
                                              name="dc", bufs=2)
                    ve.tensor_scalar_mul(
                        DC[(g, t)][:, hh * P:(hh + 1) * P], identb[:], cc_[:])
                    st["pexp"] = pexp
                    st["drec"] = drec
                    if DBG and i < 2:
                        pxs = stp.tile([P, LW[1]], FP, tag="pxs",
                                       name="pxs", bufs=2)
                        nc.scalar.copy(pxs[:, 0:LW[t]], pexp[:])
                        off = 0 if i == 0 else LW[0]
                        nc.sync.dma_start(out=dbg_px[:, off:off + LW[t]],
                                          in_=pxs[:, 0:LW[t]])
                        scs = stp.tile([P, 4], FP, tag="scs", name="scs",
                                       bufs=2)
                        nc.vector.tensor_copy(scs[:, 0:1], negmax[:])
                        nc.vector.tensor_copy(scs[:, 1:2], em[:])
                        nc.vector.tensor_copy(scs[:, 2:3], ssum2[:])
                        nc.vector.tensor_copy(scs[:, 3:4], rec[:])
                        nc.sync.dma_start(out=dbg_sc[:, 4 * i:4 * i + 4],
                                          in_=scs[:])

                def stage_pt(i):
                    h, t = i // NQT, i % NQT
                    g, hh = h // GH, h % GH
                    st = S.pop(i)
                    pexp, drec = st["pexp"], st["drec"]
                    nmt1 = NMT[t] + 1
                    if (g, t) not in PTD:
                        PTD[(g, t)] = ptsp.tile([P, nmt1, GH * P], BF,
                                                tag=f"PT{t}", name="PT",
                                                bufs=2)
                    PT = PTD[(g, t)]
                    for g0 in range(0, nmt1, 4):
                        ppt = ps2.tile([P, 4 * P], FP, tag="ppt",
                                       name="ppt", bufs=2)
                        for s4 in range(4):
                            # normal-mode matmul (PE transpose mode would
                            # ignore rhs): out[k,q] = pexp[q,k] * rec[q]
                            nc.tensor.matmul(
                                ppt[:, s4 * P:(s4 + 1) * P],
                                lhsT=pexp[:, (g0 + s4) * P:(g0 + s4 + 1) * P],
                                rhs=drec[:], start=True, stop=True)
                        dst = PT[:, g0:g0 + 4, hh * P:(hh + 1) * P]
                        if (g0 // 4) % 2 == 0:
                            nc.vector.tensor_copy(dst, ppt[:])
                        else:
                            nc.scalar.copy(dst, ppt[:])

                PO = {}   # per-(group, t) out^T PSUM accumulator

                def attnv_slice(g, sl):
                    # slices 0-2: t=0 (s 0-3, s 4-7, corr+drain)
                    # slices 3-7: t=1 (s 0-3, 4-7, 8-11, 12-15, corr+drain)
                    t = 0 if sl < 3 else 1
                    nmt = NMT[t]
                    base = sl if t == 0 else sl - 3
                    is_mm = (t == 0 and sl < 2) or (t == 1 and sl < 7)
                    if is_mm and base == 0:
                        PO[(g, t)] = ps2.tile([P, GH * P], FP, tag="poT",
                                              name="poT", bufs=1)
                    po = PO[(g, t)]
                    if is_mm:
                        PT = PTD[(g, t)]
                        for s in range(base * 4, base * 4 + 4):
                            lhsT = vown[:, t, :] if s == nmt \
                                else vsb[:, s, :]
                            nc.tensor.matmul(po[:], lhsT=lhsT,
                                             rhs=PT[:, s, :],
                                             start=(s == 0), stop=False)
                    else:
                        nc.tensor.matmul(po[:], lhsT=stail[:, t, :],
                                         rhs=DC.pop((g, t))[:],
                                         start=False, stop=True)
                        del PTD[(g, t)]
                        del PO[(g, t)]
                        dst = oT[:, g * GH:(g + 1) * GH,
                                 t * P:(t + 1) * P]
                        if t == 0:
                            nc.scalar.copy(dst, po[:])
                        else:
                            nc.vector.tensor_copy(dst, po[:])

                # prologue: qproj h0-11 mm's fill the k-mesh window
                for h in range(12):
                    qproj_mm(h)
                unpack = VCTX
                STEPS = NT + 2
                for i in range(STEPS):
                    if i == 1:
                        unpack(vsb[:, 0:4, :], 0, 4, 2)
                        unpack(vsb[:, 4:8, :], 4, 8, 2)
                        unpack(vsb[:, 8:12, :], 0, 4, 3)
                        unpack(vsb[:, 12:16, :], 4, 8, 3)
                    if i == 4:
                        # stail[t] = sum_{j > global row} v_j (bf16);
                        # borrows ps2 "ppt" psum slots
                        for t_ in range(NQT):
                            psv1 = ps2.tile([1, 4 * P], FP, tag="ppt",
                                            name="psv1", bufs=2)
                            for s in range(NS):
                                nc.tensor.matmul(
                                    psv1[:, 0:DH],
                                    lhsT=hz_sb[:, t_, s:s + 1],
                                    rhs=vsb[:, s, :],
                                    start=(s == 0), stop=(s == NS - 1))
                            nc.scalar.copy(vsuf[:], psv1[:, 0:DH])
                            pst = ps2.tile([P, 4 * P], FP, tag="ppt",
                                           name="pst", bufs=2)
                            nc.tensor.matmul(pst[:, 0:DH], lhsT=lones[:],
                                             rhs=vown[:, t_, :],
                                             start=True, stop=False)
                            nc.tensor.matmul(pst[:, 0:DH], lhsT=ones1[:],
                                             rhs=vsuf[:],
                                             start=False, stop=True,
                                             skip_group_check=True)
                            nc.vector.tensor_copy(stail[:, t_, :],
                                                  pst[:, 0:DH])
                    if i + 6 < NT:
                        pb_prefetch(i + 6, nc.gpsimd, nc.gpsimd)
                    if i < NT:
                        h, t = i // NQT, i % NQT
                        if t == 0:
                            if h + 12 < H:
                                qproj_dma(h + 12, nc.scalar if (h % 2 == 0)
                                          else nc.gpsimd)
                            if 12 <= h + 11 < H:
                                qproj_mm(h + 11)
                        stage_mm(i)
                        stage_exp(i)
                    if 0 <= i - 2 < NT:
                        stage_pt(i - 2)
                    j = i - 10
                    if j >= 0 and j // 8 < NT // 8:
                        attnv_slice(j // 8, j % 8)
                    if i < NT and i % 4 == 1 and i // 4 < WOPF:
                        hw_ = i // 4
                        nc.gpsimd.dma_start(
                            out=wo_sb[:, hw_, :],
                            in_=wo_d[hw_ * DH:(hw_ + 1) * DH, :])
                for j in range(STEPS - 10, NT):
                    attnv_slice(j // 8, j % 8)

        if DBG:
            with tc.tile_pool(name="dbgp", bufs=2) as dbgp:
                def dump(dst, src_ap, width):
                    stg = dbgp.tile([P, width], FP, tag="dstg",
                                    name="dstg", bufs=2)
                    nc.scalar.copy(stg[:], src_ap)
                    nc.sync.dma_start(out=dst[:, 0:width], in_=stg[:])
                dump(dbg_qT, qT[:, :, :], H * MQ)
                dump(dbg_kT, kTB[:], LW[1])
                dump(dbg_vsb, vsb[:, :, :], NS * P)
                dump(dbg_st, stail[:, :, :], NQT * DH)
                dump(dbg_oT, oT[:, :, :], H * MQ)

        # ---- phase 4: output projection (bf16, JIT wo streaming) ---------
        with tc.tile_pool(name="osb", bufs=2) as osbp, \
             tc.tile_pool(name="wof", bufs=1) as wofp, \
             tc.tile_pool(name="ps_out", bufs=NQT * (DIM // 512),
                          space="PSUM") as ps_out:
            pouts = []
            for t in range(NQT):
                for nk in range(DIM // 512):
                    pouts.append(ps_out.tile([P, 512], FP, tag="pout",
                                             name=f"pout{t}_{nk}"))
            # stream the non-prefetched wo heads all at once, split across
            # issue queues, so the h>=WOPF matmuls are never DMA-paced
            wfs = {}
            weng = [nc.gpsimd, nc.scalar, nc.sync]
            for h in range(WOPF, H):
                wo_b = wofp.tile([P, DIM], BF, tag=f"wof{h}", name="wo_b")
                for q2 in range(2):
                    weng[(h + q2) % 3].dma_start(
                        out=wo_b[:, q2 * 1024:(q2 + 1) * 1024],
                        in_=wo_d[h * DH:(h + 1) * DH,
                                 q2 * 1024:(q2 + 1) * 1024])
                wfs[h] = wo_b
            # chunk 7 aliases the PSUM bank the pipeline's poT holds until
            # the flush's last drain - defer it so the h-major stream never
            # head-of-line blocks on that bank; drain each chunk right after
            # its last accumulation so stores overlap remaining compute
            nwk = DIM // 512
            osbs = {}
            for t in range(NQT):
                osbs[t] = osbp.tile([P, DIM], FP, tag="osb", name="osb")

            def pout_mm(c2, h):
                t, nk = c2 // nwk, c2 % nwk
                wo_h = wo_sb[:, h, :] if h < WOPF else wfs[h][:]
                nc.tensor.matmul(pouts[c2][:],
                                 lhsT=oT[:, h, t * P:(t + 1) * P],
                                 rhs=wo_h[:, nk * 512:(nk + 1) * 512],
                                 start=(h == 0), stop=(h == H - 1))

            def pout_drain(c2):
                t, nk = c2 // nwk, c2 % nwk
                eng = nc.scalar if c2 % 2 == 0 else nc.vector
                if c2 % 2 == 0:
                    nc.scalar.copy(osbs[t][:, nk * 512:(nk + 1) * 512],
                                   pouts[c2][:])
                else:
                    nc.vector.tensor_copy(
                        osbs[t][:, nk * 512:(nk + 1) * 512], pouts[c2][:])
                nc.sync.dma_start(
                    out=out_d[t * P:(t + 1) * P, nk * 512:(nk + 1) * 512],
                    in_=osbs[t][:, nk * 512:(nk + 1) * 512])
            for h in range(H):
                for c2 in range(7):
                    pout_mm(c2, h)
                if h == H - 1:
                    for c2 in range(7):
                        pout_drain(c2)
            for h in range(H):
                pout_mm(7, h)
            pout_drain(7)

    nc.compile()
    return nc


_NC = None


def kernel(**inputs):
    global _NC, last_exec_time_ns
    x = np.asarray(inputs["x"], dtype=np.float32)[0]          # [SEQ, DIM]
    pos = np.asarray(inputs["pos_bias"], dtype=np.float32)    # [H, SEQ, SEQ]
    gamma = np.asarray(inputs["gamma"], dtype=np.float32)
    wq = np.ascontiguousarray(np.asarray(inputs["wq"], dtype=np.float32))
    wk = np.ascontiguousarray(np.asarray(inputs["wk"], dtype=np.float32))
    wv = np.ascontiguousarray(np.asarray(inputs["wv"], dtype=np.float32))
    wo = np.ascontiguousarray(np.asarray(inputs["wo"], dtype=np.float32))

    if _NC is None:
        _NC = build()

    x = np.ascontiguousarray(x)
    # device layouts: wq [P, h, c, d], wk/wv [P, c, d]; gamma folded in
    g_ = gamma[:, None]
    wqs = np.ascontiguousarray(
        (wq * g_ * np.float32(SCALE)).reshape(CD, P, H, DH)
        .transpose(1, 2, 0, 3).reshape(P, H * CD * DH))
    wk_r = np.ascontiguousarray(
        (wk * g_).reshape(CD, P, DH).transpose(1, 0, 2).reshape(P, CD * DH))
    wv_r = np.ascontiguousarray(
        (wv * g_).reshape(CD, P, DH).transpose(1, 0, 2).reshape(P, CD * DH))
    wo_b = np.ascontiguousarray(wo.astype(ml_dtypes.bfloat16))
    NEG = np.float32(-1e9)
    jidx = np.arange(SEQ)
    jl = np.arange(P)
    pidx = np.arange(P)
    triu_strict = (jl[None, :] > pidx[:, None])               # [row, col]
    lones = np.ascontiguousarray(triu_strict.T.astype(ml_dtypes.bfloat16))
    in_maps = []
    for m in range(N_CORES):
        pbs = {}
        for t in range(NQT):
            g = m + 8 * t
            gq = slice(P * g, P * (g + 1))
            nm = NMT[t]
            pb = np.empty((H, P, LW[t]), np.float32)
            # main region: global cols [0 : nm*P], visible iff j < P*g
            pb[:, :, :nm * P] = np.where(
                (jidx[:nm * P] < P * g)[None, None, :],
                pos[:, gq, :nm * P], NEG)
            # diag slot: own tile with strict-upper mask
            pb[:, :, nm * P:] = np.where(
                triu_strict[None, :, :], NEG,
                pos[:, gq, P * g:P * (g + 1)])
            pbs[t] = np.ascontiguousarray(
                pb.reshape(H * P, LW[t]).astype(ml_dtypes.bfloat16))
        hz = np.zeros((P, NQT, NS), np.float32)
        cnt_m = np.zeros((P, NQT), np.float32)
        xq_m = np.zeros((MQ, DIM), np.float32)
        for t in range(NQT):
            g = m + 8 * t
            xq_m[t * P:(t + 1) * P] = x[P * g:P * (g + 1)]
            for s in range(NS):
                hz[:, t, s] = ((s * P + pidx) >= P * (g + 1)).astype(
                    np.float32)
            cnt_m[:, t] = (SEQ - 1) - (P * g + pidx)
        xq2 = (xq_m.reshape(NQT, P, 8, 256).transpose(0, 2, 1, 3)
               .reshape(NQT * 8 * P, 256))
        in_maps.append({
            "xq": np.ascontiguousarray(xq2),
            "pbA": pbs[0], "pbB": pbs[1],
            "lones": lones,
            "cnt": np.ascontiguousarray(cnt_m),
            "hz": np.ascontiguousarray(
                hz.reshape(P, NQT * NS).astype(ml_dtypes.bfloat16)),
            "wq": wqs, "wk": wk_r, "wv": wv_r, "wo": wo_b,
        })
    trace = os.environ.get("KERNEL_TRACE") == "1"
    res = run_bass_kernel_spmd(_NC, in_maps, core_ids=list(range(N_CORES)),
                               trace=trace)
    last_exec_time_ns = res.exec_time_ns
    global last_results
    last_results = res.results
    out = np.zeros((SEQ, DIM), np.float32)
    for m in range(N_CORES):
        om = res.results[m]["out"]
        out[P * m:P * (m + 1)] = om[0:P]
        out[P * (m + 8):P * (m + 9)] = om[P:MQ]
    return out[None, ...].astype(np.float32)


# revision 36
# speedup vs baseline: 1.0427x; 1.0427x over previous
"""Trainium2 Bass kernel for MQA attention with RMSNorm + positional bias.

Reference computation (per core, seq-sharded over 8 cores):
  xn = rmsnorm(x) * gamma
  q = (xn @ wq) * scale   (16 heads x 128)     k = xn @ wk    v = xn @ wv
  sim = q @ k^T + pos_bias ; masked (non-causal entries := 1e-10)
  attn = softmax(sim); out = (attn @ v, concat heads) @ wo

Sharding: core m owns query rows of global 128-row tiles {m, m+8}. K/V
(shared MQA head) are computed from own rows and AllGathered in ONE
combined collective (k fp32 cols [0:256], v fp32 cols [256:512]).

Design (v2 rewrite of the 421us baseline):
- pos_bias AND the causal mask are folded into one additive host tensor
  pb (bf16, -1e9 at masked cols incl. a packed diag slot). pb is
  accumulated into the sim PSUM by the PE itself (identity-weight bf16
  matmul, 1 cycle/row) - no elementwise mask/mult pass and no gated-k
  copies. Masked cols then vanish through exp naturally, and the row max
  (V) can read raw PSUM.
- psim chunks are exactly 512 wide (1 PSUM bank): [512,384+128diag] for
  t=0 (7 main tiles), [512,512,512,384+128diag] for t=1 (15 main tiles).
- softmax denominator comes free via exp's accum_out; the masked-value
  correction of the reference (exp(1e-10-m) per masked col) is
  cnt*em as in the baseline.
- Normalization is folded into the P^T transposes: rhs = diag(rec)
  instead of identity, so P^T arrives pre-normalized.
- attn@v runs transposed per 4-head group: out^T[d, 4*128 q] with v
  tiles as weights - output lands directly in oT layout (no per-unit
  normalize chain, no 32 output transposes), and the masked-value
  output correction em*rec*stail rides one extra matmul with
  rhs = blockdiag(c_h), c_h = em_h*rec_h.
- k/v projections accumulate chunk-by-chunk interleaved with the x^T
  transposes so the combined AllGather triggers ~20us in; qproj +
  wq streaming fill the mesh-latency window.
"""

import os

import numpy as np
import ml_dtypes

import concourse.bass as bass
import concourse.mybir as mybir
import concourse.tile as tile
from concourse import bacc, masks
from concourse.bass_utils import run_bass_kernel_spmd

SEQ = 2048
DIM = 2048
H = 16
DH = 128
P = 128
N_CORES = 8
MQ = SEQ // N_CORES      # 256 query rows per core
NQT = MQ // P            # 2 query tiles per core
CD = DIM // P            # 16 contraction chunks
NS = SEQ // P            # 16 seq tiles
NMT = {0: 7, 1: 15}      # main key tiles per unit type
NCH = {0: 2, 1: 4}       # 512-wide sim chunks per unit type (diag packed)
LW = {0: 1024, 1: 2048}  # pb / pexp width per unit type
NT = H * NQT             # 32 pipeline units
GH = 4                   # heads per attn@v group
WOPF = 8                 # wo heads prefetched during the pipeline
SCALE = DH ** -0.5
EPS = 1e-5

FP = mybir.dt.float32
F16 = mybir.dt.float16
FR = mybir.dt.float32r
BF = mybir.dt.bfloat16
AF = mybir.ActivationFunctionType
ALU = mybir.AluOpType
AX = mybir.AxisListType

last_exec_time_ns = None


def _rms_scale_rows(nc, pool, xt, tag):
    """In-place x *= rsqrt(mean(x^2)+eps) for a [P, DIM] tile."""
    sq = pool.tile([P, DIM], FP, tag="sq_scratch", name="sq_scratch", bufs=1)
    ssq = pool.tile([P, 1], FP, tag=f"ssq{tag}", name=f"ssq{tag}")
    nc.scalar.activation(sq[:], xt[:], AF.Square, accum_out=ssq[:])
    nc.vector.tensor_scalar(ssq[:], ssq[:], 1.0 / DIM, EPS, ALU.mult, ALU.add)
    nc.scalar.sqrt(ssq[:], ssq[:])
    nc.vector.reciprocal(ssq[:], ssq[:])
    nc.vector.tensor_scalar_mul(xt[:], xt[:], ssq[:])


def build():
    nc = bacc.Bacc("TRN2", target_bir_lowering=False, debug=False,
                   num_devices=N_CORES)
    xq_d = nc.dram_tensor("xq", [NQT * 8 * P, 256], FP,
                          kind="ExternalInput")
    pbA_d = nc.dram_tensor("pbA", [H * P, LW[0]], BF, kind="ExternalInput")
    pbB_d = nc.dram_tensor("pbB", [H * P, LW[1]], BF, kind="ExternalInput")
    lo_d = nc.dram_tensor("lones", [P, P], BF, kind="ExternalInput")
    cnt_d = nc.dram_tensor("cnt", [P, NQT], FP, kind="ExternalInput")
    hz_d = nc.dram_tensor("hz", [P, NQT * NS], BF, kind="ExternalInput")
    wq_d = nc.dram_tensor("wq", [P, H * CD * DH], FR, kind="ExternalInput")
    wk_d = nc.dram_tensor("wk", [P, CD * DH], FR, kind="ExternalInput")
    wv_d = nc.dram_tensor("wv", [P, CD * DH], FR, kind="ExternalInput")
    wo_d = nc.dram_tensor("wo", [H * DH, DIM], BF, kind="ExternalInput")
    out_d = nc.dram_tensor("out", [MQ, DIM], FP, kind="ExternalOutput")
    DBG = os.environ.get("KERNEL_DEBUG") == "1"
    if DBG:
        dbg_qT = nc.dram_tensor("dbg_qT", [P, H * MQ], FP,
                                kind="ExternalOutput")
        dbg_kT = nc.dram_tensor("dbg_kT", [P, SEQ], FP, kind="ExternalOutput")
        dbg_vsb = nc.dram_tensor("dbg_vsb", [P, NS * P], FP,
                                 kind="ExternalOutput")
        dbg_st = nc.dram_tensor("dbg_st", [P, NQT * DH], FP,
                                kind="ExternalOutput")
        dbg_oT = nc.dram_tensor("dbg_oT", [P, H * MQ], FP,
                                kind="ExternalOutput")
        dbg_px = nc.dram_tensor("dbg_px", [P, LW[0] + LW[1]], FP,
                                kind="ExternalOutput")
        dbg_sc = nc.dram_tensor("dbg_sc", [P, 8], FP, kind="ExternalOutput")

    with tile.TileContext(nc) as tc, \
         tc.tile_pool(name="singles", bufs=1) as singles:
        # ---- persistent tiles --------------------------------------------
        ident = singles.tile([P, P], FP, tag="ident", name="ident")
        masks.make_identity(nc, ident[:])
        identb = singles.tile([P, P], BF, tag="identb", name="identb")
        masks.make_identity(nc, identb[:])
        cnt = singles.tile([P, NQT], FP, tag="cnt", name="cnt")

        qT = singles.tile([P, H, MQ], FR, tag="qT", name="qT")
        kTA = singles.tile([P, LW[0]], FR, tag="kTA", name="kTA")
        kTB = singles.tile([P, LW[1]], FR, tag="kTB", name="kTB")
        vsb = singles.tile([P, NS, P], BF, tag="vsb", name="vsb")
        vown = singles.tile([P, NQT, P], BF, tag="vown", name="vown")
        stail = singles.tile([P, NQT, DH], BF, tag="stail", name="stail")
        oT = singles.tile([P, H, MQ], BF, tag="oT", name="oT")
        wo_sb = singles.tile([P, WOPF, DIM], BF, tag="wo_sb", name="wo_sb")
        hz_sb = singles.tile([P, NQT, NS], BF, tag="hz", name="hz_sb")
        lones = singles.tile([P, P], BF, tag="lones", name="lones")
        ones1 = singles.tile([1, P], BF, tag="ones1", name="ones1")
        vsuf = singles.tile([1, DH], BF, tag="vsuf", name="vsuf")

        with tc.tile_pool(name="xnTqp", bufs=1) as xnTqp, \
             tc.tile_pool(name="pos", bufs=1) as posp, \
             tc.tile_pool(name="wqp", bufs=3) as wqp:
            xnT = xnTqp.tile([P, CD, MQ], FR, tag="xnT", name="xnT")
            WQH = {}
            PB = {}

            def qproj_dma(h, eng):
                wqh = wqp.tile([P, CD, DH], FR, tag="wqh",
                               name="wqh", bufs=3)
                for cb in range(4):
                    eng.dma_start(
                        out=wqh[:, cb * 4:(cb + 1) * 4, :],
                        in_=wq_d[:, h * CD * DH + cb * 512:
                                 h * CD * DH + (cb + 1) * 512])
                WQH[h] = wqh

            def pb_prefetch(i, eng0, eng1):
                h, t = i // NQT, i % NQT
                pbsel = pbA_d if t == 0 else pbB_d
                pb_t = posp.tile([P, LW[t]], BF, tag=f"pb{t}",
                                 name="pb", bufs=3)
                half = LW[t] // 2
                eng0.dma_start(out=pb_t[:, 0:half],
                               in_=pbsel[h * P:(h + 1) * P, 0:half])
                eng1.dma_start(out=pb_t[:, half:],
                               in_=pbsel[h * P:(h + 1) * P, half:])
                PB[i] = pb_t

            # ---- phase 0+1: xn^T, k/v proj, combined AllGather -----------
            with tc.tile_pool(name="ph0", bufs=1) as ph0, \
                 tc.tile_pool(name="kvw", bufs=1) as kvwp, \
                 tc.tile_pool(name="dram", bufs=1, space="DRAM") as dramp, \
                 tc.tile_pool(name="pstr0", bufs=2, space="PSUM") as pstr0, \
                 tc.tile_pool(name="psk", bufs=1, space="PSUM") as psk, \
                 tc.tile_pool(name="psv", bufs=1, space="PSUM") as psv, \
                 tc.tile_pool(name="pstv", bufs=1, space="PSUM") as pstv:
                wk_sb = kvwp.tile([P, CD, DH], FR, tag="wk", name="wk_sb")
                wv_sb = kvwp.tile([P, CD, DH], FR, tag="wv", name="wv_sb")
                # x load first: 8 column chunks spread over issue queues
                xnq = []
                engs = [nc.sync, nc.scalar, nc.gpsimd]
                ei = 0
                for t in range(NQT):
                    xt = ph0.tile([P, DIM], FP, tag=f"xq{t}", name=f"xq{t}")
                    for ci in range(8):
                        blk = (t * 8 + ci) * P
                        engs[ei % 3].dma_start(
                            out=xt[:, ci * 256:(ci + 1) * 256],
                            in_=xq_d[blk:blk + P, :])
                        ei += 1
                    xnq.append(xt)
                for ci in range(4):
                    nc.scalar.dma_start(
                        out=wk_sb[:, ci * 4:(ci + 1) * 4, :],
                        in_=wk_d[:, ci * 512:(ci + 1) * 512])
                    nc.gpsimd.dma_start(
                        out=wv_sb[:, ci * 4:(ci + 1) * 4, :],
                        in_=wv_d[:, ci * 512:(ci + 1) * 512])
                nc.vector.memset(ones1[:], 1.0)
                nc.scalar.dma_start(out=lones[:], in_=lo_d[:])
                nc.scalar.dma_start(out=hz_sb[:], in_=hz_d[:])
                nc.sync.dma_start(out=cnt[:], in_=cnt_d[:])
                for t in range(NQT):
                    _rms_scale_rows(nc, ph0, xnq[t], f"q{t}")
                # transposes interleaved with k/v projection accumulation
                # (gamma is folded into wq/wk/wv host-side)
                pk = psk.tile([P, MQ], FP, tag="pk", name="pk")
                pv = psv.tile([P, MQ], FP, tag="pv", name="pv")
                # t0 transposes stream while t1's x still loads; k/v proj
                # accumulate full-width (fp32r needs free>=256 for 1c/row)
                for c in range(CD):
                    pt = pstr0.tile([P, P], FP, tag="trq", name="trq")
                    nc.tensor.transpose(pt[:], xnq[0][:, c * P:(c + 1) * P],
                                        ident[:])
                    if c % 2 == 0:
                        nc.vector.tensor_copy(xnT[:, c, 0:P], pt[:])
                    else:
                        nc.scalar.copy(xnT[:, c, 0:P], pt[:])
                for c in range(CD + 1):
                    if c < CD:
                        pt = pstr0.tile([P, P], FP, tag="trq", name="trq")
                        nc.tensor.transpose(
                            pt[:], xnq[1][:, c * P:(c + 1) * P], ident[:])
                        if c % 2 == 0:
                            nc.vector.tensor_copy(xnT[:, c, P:MQ], pt[:])
                        else:
                            nc.scalar.copy(xnT[:, c, P:MQ], pt[:])
                    if c > 0:
                        nc.tensor.matmul(pk[:], lhsT=wk_sb[:, c - 1, :],
                                         rhs=xnT[:, c - 1, :],
                                         start=(c == 1), stop=(c == CD))
                        nc.tensor.matmul(pv[:], lhsT=wv_sb[:, c - 1, :],
                                         rhs=xnT[:, c - 1, :],
                                         start=(c == 1), stop=(c == CD))
                # k bounce (fp32) and v bounce (bf16, = vown layout)
                kbsb = kvwp.tile([P, MQ], F16, tag="kbsb", name="kbsb")
                nc.scalar.copy(kbsb[:], pk[:])
                vTs = kvwp.tile([P, MQ], FP, tag="vTs", name="vTs")
                nc.vector.tensor_copy(vTs[:], pv[:])
                for t in range(NQT):
                    ptv = pstv.tile([P, P], FP, tag="vtr", name="vtr")
                    nc.tensor.transpose(ptv[:], vTs[:, t * P:(t + 1) * P],
                                        ident[:])
                    nc.vector.tensor_copy(vown[:, t, :], ptv[:])
                # fused k(fp16-bits)+v(bf16) bounce: ONE mesh
                kv_bounce = dramp.tile([P, 2 * MQ], BF, tag="kvb",
                                       name="kv_bounce")
                kv_ag = dramp.tile([N_CORES, P, 2 * MQ], BF, tag="kvag",
                                   name="kv_ag", addr_space="Shared")
                nc.scalar.dma_start(out=kv_bounce[:, 0:MQ],
                                    in_=kbsb[:].bitcast(BF))
                nc.scalar.dma_start(out=kv_bounce[:, MQ:2 * MQ],
                                    in_=vown[:, :, :])
                rg = [list(range(N_CORES))]
                nc.gpsimd.collective_compute(
                    "AllGather", ALU.bypass, replica_groups=rg,
                    ins=[kv_bounce[:].opt()], outs=[kv_ag[:, :, :].opt()])
                # k unpack into per-t layouts (wide permuted-AP DMAs on the
                # gpsimd queue, which is dead between the two collectives)
                kstgA = kvwp.tile([P, LW[0]], F16, tag="kstgA",
                                  name="kstgA")
                kstgB = kvwp.tile([P, LW[1]], F16, tag="kstgB",
                                  name="kstgB")
                nc.scalar.copy(kstgA[:, NMT[0] * P:], pk[:, 0:P])
                nc.scalar.copy(kstgB[:, NMT[1] * P:], pk[:, P:MQ])

                def unpack(dst, r0, r1, h0, cast=None):
                    in_ = kv_ag[r0:r1, :, h0 * P:(h0 + 1) * P] \
                        .transpose([1, 0, 2])
                    if cast is not None:
                        in_ = in_.bitcast(cast)
                    nc.gpsimd.dma_start(out=dst, in_=in_)
                unpack(kstgA[:, 0:3 * P], 0, 3, 0, F16)
                unpack(kstgA[:, 3 * P:5 * P], 3, 5, 0, F16)
                unpack(kstgA[:, 5 * P:7 * P], 5, 7, 0, F16)
                nc.vector.tensor_copy(kTA[:], kstgA[:])
                unpack(kstgB[:, 0:4 * P], 0, 4, 0, F16)
                unpack(kstgB[:, 4 * P:8 * P], 4, 8, 0, F16)
                unpack(kstgB[:, 8 * P:12 * P], 0, 4, 1, F16)
                unpack(kstgB[:, 12 * P:15 * P], 4, 7, 1, F16)
                nc.vector.tensor_copy(kTB[:], kstgB[:])
                # prologue prefetches at the tail of phase-1's scalar queue:
                # transfers start only once the kproj path stops needing HBM
                for h in range(12):
                    qproj_dma(h, nc.scalar)
                for i in range(6):
                    pb_prefetch(i, nc.scalar, nc.scalar)
                VCTX = unpack

            # ---- phase 2: q proj + attention pipeline --------------------
            with tc.tile_pool(name="pexpp", bufs=1) as pexpp, \
                 tc.tile_pool(name="pts", bufs=1) as ptsp, \
                 tc.tile_pool(name="st", bufs=1) as stp, \
                 tc.tile_pool(name="ps2", bufs=1, space="PSUM") as ps2:

                def qproj_mm(h):
                    wqh = WQH.pop(h)
                    pq = ps2.tile([P, MQ], FP, tag="psim", name="pq",
                                  bufs=5)
                    for c in range(CD):
                        nc.tensor.matmul(pq[:], lhsT=wqh[:, c, :],
                                         rhs=xnT[:, c, :],
                                         start=(c == 0), stop=(c == CD - 1))
                    nc.scalar.copy(qT[:, h, :], pq[:])

                S = {}    # per-unit pipeline state
                PTD = {}  # per-(group, t) P^T staging tiles
                DC = {}   # per-(group, t) blockdiag(em*rec) tiles

                def stage_mm(i):
                    h, t = i // NQT, i % NQT
                    nch = NCH[t]
                    pb_t = PB.pop(i)
                    kTt = kTA if t == 0 else kTB
                    qsl = qT[:, h, t * P:(t + 1) * P]
                    psims = []
                    for c in range(nch):
                        psim = ps2.tile([P, 512], FP, tag="psim",
                                        name="psim", bufs=5)
                        nc.tensor.matmul(psim[:], lhsT=qsl,
                                         rhs=kTt[:, c * 512:(c + 1) * 512],
                                         start=True, stop=False)
                        nc.tensor.matmul(psim[:], lhsT=identb[:],
                                         rhs=pb_t[:, c * 512:(c + 1) * 512],
                                         start=False, stop=True)
                        psims.append(psim)
                    mxc = stp.tile([P, 4], FP, tag="mxc", name="mxc",
                                   bufs=3)
                    for c in range(nch):
                        nc.vector.tensor_reduce(mxc[:, c:c + 1], psims[c][:],
                                                axis=AX.X, op=ALU.max)
                    negmax = stp.tile([P, 1], FP, tag="negmax",
                                      name="negmax", bufs=4)
                    nc.vector.tensor_reduce(negmax[:], mxc[:, 0:nch],
                                            axis=AX.X, op=ALU.max,
                                            negate=True)
                    nc.vector.tensor_scalar(negmax[:], negmax[:], 1.0, 0.0,
                                            ALU.mult, ALU.min)
                    S[i] = {"psims": psims, "negmax": negmax}

                def stage_exp(i):
                    h, t = i // NQT, i % NQT
                    g, hh = h // GH, h % GH
                    nch = NCH[t]
                    st = S[i]
                    negmax = st["negmax"]
                    pexp = pexpp.tile([P, LW[t]], BF, tag=f"pexp{t}",
                                      name="pexp", bufs=3)
                    sexp = stp.tile([P, 4], FP, tag="sexp", name="sexp",
                                    bufs=3)
                    for c in range(nch):
                        nc.scalar.activation(pexp[:, c * 512:(c + 1) * 512],
                                             st["psims"][c][:], AF.Exp,
                                             bias=negmax[:],
                                             accum_out=sexp[:, c:c + 1])
                    em = stp.tile([P, 1], FP, tag="em", name="em", bufs=4)
                    nc.scalar.activation(em[:], negmax[:], AF.Exp)
                    ssum = stp.tile([P, 1], FP, tag="ssum", name="ssum",
                                    bufs=4)
                    nc.vector.tensor_reduce(ssum[:], sexp[:, 0:nch],
                                            axis=AX.X, op=ALU.add)
                    ve = nc.vector
                    ssum2 = stp.tile([P, 1], FP, tag="ssum2", name="ssum2",
                                     bufs=4)
                    nc.vector.scalar_tensor_tensor(
                        ssum2[:], cnt[:, t:t + 1], em[:], ssum[:],
                        op0=ALU.mult, op1=ALU.add)
                    rec = stp.tile([P, 1], FP, tag="rec", name="rec", bufs=4)
                    nc.vector.reciprocal(rec[:], ssum2[:])
                    drec = stp.tile([P, P], BF, tag="drec", name="drec",
                                    bufs=4)
                    ve.tensor_scalar_mul(drec[:], identb[:], rec[:])
                    cc_ = stp.tile([P, 1], FP, tag="cc", name="cc", bufs=4)
                    ve.tensor_tensor(cc_[:], em[:], rec[:], op=ALU.mult)
                    if (g, t) not in DC:
                        DC[(g, t)] = stp.tile([P, GH * P], BF, tag=f"dc{t}",
                                              name="dc", bufs=2)
                    ve.tensor_scalar_mul(
                        DC[(g, t)][:, hh * P:(hh + 1) * P], identb[:], cc_[:])
                    st["pexp"] = pexp
                    st["drec"] = drec
                    if DBG and i < 2:
                        pxs = stp.tile([P, LW[1]], FP, tag="pxs",
                                       name="pxs", bufs=2)
                        nc.scalar.copy(pxs[:, 0:LW[t]], pexp[:])
                        off = 0 if i == 0 else LW[0]
                        nc.sync.dma_start(out=dbg_px[:, off:off + LW[t]],
                                          in_=pxs[:, 0:LW[t]])
                        scs = stp.tile([P, 4], FP, tag="scs", name="scs",
                                       bufs=2)
                        nc.vector.tensor_copy(scs[:, 0:1], negmax[:])
                        nc.vector.tensor_copy(scs[:, 1:2], em[:])
                        nc.vector.tensor_copy(scs[:, 2:3], ssum2[:])
                        nc.vector.tensor_copy(scs[:, 3:4], rec[:])
                        nc.sync.dma_start(out=dbg_sc[:, 4 * i:4 * i + 4],
                                          in_=scs[:])

                def stage_pt(i):
                    h, t = i // NQT, i % NQT
                    g, hh = h // GH, h % GH
                    st = S.pop(i)
                    pexp, drec = st["pexp"], st["drec"]
                    nmt1 = NMT[t] + 1
                    if (g, t) not in PTD:
                        PTD[(g, t)] = ptsp.tile([P, nmt1, GH * P], BF,
                                                tag=f"PT{t}", name="PT",
                                                bufs=2)
                    PT = PTD[(g, t)]
                    for g0 in range(0, nmt1, 4):
                        ppt = ps2.tile([P, 4 * P], FP, tag="ppt",
                                       name="ppt", bufs=2)
                        for s4 in range(4):
                            # normal-mode matmul (PE transpose mode would
                            # ignore rhs): out[k,q] = pexp[q,k] * rec[q]
                            nc.tensor.matmul(
                                ppt[:, s4 * P:(s4 + 1) * P],
                                lhsT=pexp[:, (g0 + s4) * P:(g0 + s4 + 1) * P],
                                rhs=drec[:], start=True, stop=True)
                        dst = PT[:, g0:g0 + 4, hh * P:(hh + 1) * P]
                        if (g0 // 4) % 2 == 0:
                            nc.vector.tensor_copy(dst, ppt[:])
                        else:
                            nc.scalar.copy(dst, ppt[:])

                PO = {}   # per-(group, t) out^T PSUM accumulator

                def attnv_slice(g, sl):
                    # slices 0-2: t=0 (s 0-3, s 4-7, corr+drain)
                    # slices 3-7: t=1 (s 0-3, 4-7, 8-11, 12-15, corr+drain)
                    t = 0 if sl < 3 else 1
                    nmt = NMT[t]
                    base = sl if t == 0 else sl - 3
                    is_mm = (t == 0 and sl < 2) or (t == 1 and sl < 7)
                    if is_mm and base == 0:
                        PO[(g, t)] = ps2.tile([P, GH * P], FP, tag="poT",
                                              name="poT", bufs=1)
                    po = PO[(g, t)]
                    if is_mm:
                        PT = PTD[(g, t)]
                        for s in range(base * 4, base * 4 + 4):
                            lhsT = vown[:, t, :] if s == nmt \
                                else vsb[:, s, :]
                            nc.tensor.matmul(po[:], lhsT=lhsT,
                                             rhs=PT[:, s, :],
                                             start=(s == 0), stop=False)
                    else:
                        nc.tensor.matmul(po[:], lhsT=stail[:, t, :],
                                         rhs=DC.pop((g, t))[:],
                                         start=False, stop=True)
                        del PTD[(g, t)]
                        del PO[(g, t)]
                        dst = oT[:, g * GH:(g + 1) * GH,
                                 t * P:(t + 1) * P]
                        if t == 0:
                            nc.scalar.copy(dst, po[:])
                        else:
                            nc.vector.tensor_copy(dst, po[:])

                # prologue: qproj h0-11 mm's fill the k-mesh window
                for h in range(12):
                    qproj_mm(h)
                unpack = VCTX
                STEPS = NT + 2
                for i in range(STEPS):
                    if i == 1:
                        unpack(vsb[:, 0:4, :], 0, 4, 2)
                        unpack(vsb[:, 4:8, :], 4, 8, 2)
                        unpack(vsb[:, 8:12, :], 0, 4, 3)
                        unpack(vsb[:, 12:16, :], 4, 8, 3)
                    if i == 4:
                        # stail[t] = sum_{j > global row} v_j (bf16);
                        # borrows ps2 "ppt" psum slots
                        for t_ in range(NQT):
                            psv1 = ps2.tile([1, 4 * P], FP, tag="ppt",
                                            name="psv1", bufs=2)
                            for s in range(NS):
                                nc.tensor.matmul(
                                    psv1[:, 0:DH],
                                    lhsT=hz_sb[:, t_, s:s + 1],
                                    rhs=vsb[:, s, :],
                                    start=(s == 0), stop=(s == NS - 1))
                            nc.scalar.copy(vsuf[:], psv1[:, 0:DH])
                            pst = ps2.tile([P, 4 * P], FP, tag="ppt",
                                           name="pst", bufs=2)
                            nc.tensor.matmul(pst[:, 0:DH], lhsT=lones[:],
                                             rhs=vown[:, t_, :],
                                             start=True, stop=False)
                            nc.tensor.matmul(pst[:, 0:DH], lhsT=ones1[:],
                                             rhs=vsuf[:],
                                             start=False, stop=True,
                                             skip_group_check=True)
                            nc.vector.tensor_copy(stail[:, t_, :],
                                                  pst[:, 0:DH])
                    if i + 6 < NT:
                        pb_prefetch(i + 6, nc.gpsimd, nc.gpsimd)
                    if i < NT:
                        h, t = i // NQT, i % NQT
                        if t == 0:
                            if h + 12 < H:
                                qproj_dma(h + 12, nc.scalar if (h % 2 == 0)
                                          else nc.gpsimd)
                            if 12 <= h + 11 < H:
                                qproj_mm(h + 11)
                        stage_mm(i)
                        stage_exp(i)
                    if 0 <= i - 2 < NT:
                        stage_pt(i - 2)
                    j = i - 10
                    if j >= 0 and j // 8 < NT // 8:
                        attnv_slice(j // 8, j % 8)
                    if i < NT and i % 4 == 1 and i // 4 < WOPF:
                        hw_ = i // 4
                        nc.gpsimd.dma_start(
                            out=wo_sb[:, hw_, :],
                            in_=wo_d[hw_ * DH:(hw_ + 1) * DH, :])
                for j in range(STEPS - 10, NT):
                    attnv_slice(j // 8, j % 8)

        if DBG:
            with tc.tile_pool(name="dbgp", bufs=2) as dbgp:
                def dump(dst, src_ap, width):
                    stg = dbgp.tile([P, width], FP, tag="dstg",
                                    name="dstg", bufs=2)
                    nc.scalar.copy(stg[:], src_ap)
                    nc.sync.dma_start(out=dst[:, 0:width], in_=stg[:])
                dump(dbg_qT, qT[:, :, :], H * MQ)
                dump(dbg_kT, kTB[:], LW[1])
                dump(dbg_vsb, vsb[:, :, :], NS * P)
                dump(dbg_st, stail[:, :, :], NQT * DH)
                dump(dbg_oT, oT[:, :, :], H * MQ)

        # ---- phase 4: output projection (bf16, JIT wo streaming) ---------
        with tc.tile_pool(name="osb", bufs=2) as osbp, \
             tc.tile_pool(name="wof", bufs=1) as wofp, \
             tc.tile_pool(name="ps_out", bufs=NQT * (DIM // 512),
                          space="PSUM") as ps_out:
            pouts = []
            for t in range(NQT):
                for nk in range(DIM // 512):
                    pouts.append(ps_out.tile([P, 512], FP, tag="pout",
                                             name=f"pout{t}_{nk}"))
            # stream the non-prefetched wo heads all at once, split across
            # issue queues, so the h>=WOPF matmuls are never DMA-paced
            wfs = {}
            weng = [nc.gpsimd, nc.scalar, nc.sync]
            for h in range(WOPF, H):
                wo_b = wofp.tile([P, DIM], BF, tag=f"wof{h}", name="wo_b")
                for q2 in range(2):
                    weng[(h + q2) % 3].dma_start(
                        out=wo_b[:, q2 * 1024:(q2 + 1) * 1024],
                        in_=wo_d[h * DH:(h + 1) * DH,
                                 q2 * 1024:(q2 + 1) * 1024])
                wfs[h] = wo_b
            # chunk 7 aliases the PSUM bank the pipeline's poT holds until
            # the flush's last drain - defer it so the h-major stream never
            # head-of-line blocks on that bank; drain each chunk right after
            # its last accumulation so stores overlap remaining compute
            nwk = DIM // 512
            osbs = {}
            for t in range(NQT):
                osbs[t] = osbp.tile([P, DIM], FP, tag="osb", name="osb")

            def pout_mm(c2, h):
                t, nk = c2 // nwk, c2 % nwk
                wo_h = wo_sb[:, h, :] if h < WOPF else wfs[h][:]
                nc.tensor.matmul(pouts[c2][:],
                                 lhsT=oT[:, h, t * P:(t + 1) * P],
                                 rhs=wo_h[:, nk * 512:(nk + 1) * 512],
                                 start=(h == 0), stop=(h == H - 1))

            def pout_drain(c2):
                t, nk = c2 // nwk, c2 % nwk
                eng = nc.scalar if c2 % 2 == 0 else nc.vector
                if c2 % 2 == 0:
                    nc.scalar.copy(osbs[t][:, nk * 512:(nk + 1) * 512],
                                   pouts[c2][:])
                else:
                    nc.vector.tensor_copy(
                        osbs[t][:, nk * 512:(nk + 1) * 512], pouts[c2][:])
                nc.sync.dma_start(
                    out=out_d[t * P:(t + 1) * P, nk * 512:(nk + 1) * 512],
                    in_=osbs[t][:, nk * 512:(nk + 1) * 512])
            for h in range(H):
                for c2 in range(7):
                    pout_mm(c2, h)
                if h == H - 1:
                    for c2 in range(7):
                        pout_drain(c2)
            for h in range(H):
                pout_mm(7, h)
            pout_drain(7)

    nc.compile()
    return nc


_NC = None


def kernel(**inputs):
    global _NC, last_exec_time_ns
    x = np.asarray(inputs["x"], dtype=np.float32)[0]          # [SEQ, DIM]
    pos = np.asarray(inputs["pos_bias"], dtype=np.float32)    # [H, SEQ, SEQ]
    gamma = np.asarray(inputs["gamma"], dtype=np.float32)
    wq = np.ascontiguousarray(np.asarray(inputs["wq"], dtype=np.float32))
    wk = np.ascontiguousarray(np.asarray(inputs["wk"], dtype=np.float32))
    wv = np.ascontiguousarray(np.asarray(inputs["wv"], dtype=np.float32))
    wo = np.ascontiguousarray(np.asarray(inputs["wo"], dtype=np.float32))

    if _NC is None:
        _NC = build()

    x = np.ascontiguousarray(x)
    # device layouts: wq [P, h, c, d], wk/wv [P, c, d]; gamma folded in
    g_ = gamma[:, None]
    wqs = np.ascontiguousarray(
        (wq * g_ * np.float32(SCALE)).reshape(CD, P, H, DH)
        .transpose(1, 2, 0, 3).reshape(P, H * CD * DH))
    wk_r = np.ascontiguousarray(
        (wk * g_).reshape(CD, P, DH).transpose(1, 0, 2).reshape(P, CD * DH))
    wv_r = np.ascontiguousarray(
        (wv * g_).reshape(CD, P, DH).transpose(1, 0, 2).reshape(P, CD * DH))
    wo_b = np.ascontiguousarray(wo.astype(ml_dtypes.bfloat16))
    NEG = np.float32(-1e9)
    jidx = np.arange(SEQ)
    jl = np.arange(P)
    pidx = np.arange(P)
    triu_strict = (jl[None, :] > pidx[:, None])               # [row, col]
    lones = np.ascontiguousarray(triu_strict.T.astype(ml_dtypes.bfloat16))
    in_maps = []
    for m in range(N_CORES):
        pbs = {}
        for t in range(NQT):
            g = m + 8 * t
            gq = slice(P * g, P * (g + 1))
            nm = NMT[t]
            pb = np.empty((H, P, LW[t]), np.float32)
            # main region: global cols [0 : nm*P], visible iff j < P*g
            pb[:, :, :nm * P] = np.where(
                (jidx[:nm * P] < P * g)[None, None, :],
                pos[:, gq, :nm * P], NEG)
            # diag slot: own tile with strict-upper mask
            pb[:, :, nm * P:] = np.where(
                triu_strict[None, :, :], NEG,
                pos[:, gq, P * g:P * (g + 1)])
            pbs[t] = np.ascontiguousarray(
                pb.reshape(H * P, LW[t]).astype(ml_dtypes.bfloat16))
        hz = np.zeros((P, NQT, NS), np.float32)
        cnt_m = np.zeros((P, NQT), np.float32)
        xq_m = np.zeros((MQ, DIM), np.float32)
        for t in range(NQT):
            g = m + 8 * t
            xq_m[t * P:(t + 1) * P] = x[P * g:P * (g + 1)]
            for s in range(NS):
                hz[:, t, s] = ((s * P + pidx) >= P * (g + 1)).astype(
                    np.float32)
            cnt_m[:, t] = (SEQ - 1) - (P * g + pidx)
        xq2 = (xq_m.reshape(NQT, P, 8, 256).transpose(0, 2, 1, 3)
               .reshape(NQT * 8 * P, 256))
        in_maps.append({
            "xq": np.ascontiguousarray(xq2),
            "pbA": pbs[0], "pbB": pbs[1],
            "lones": lones,
            "cnt": np.ascontiguousarray(cnt_m),
            "hz": np.ascontiguousarray(
                hz.reshape(P, NQT * NS).astype(ml_dtypes.bfloat16)),
            "wq": wqs, "wk": wk_r, "wv": wv_r, "wo": wo_b,
        })
    trace = os.environ.get("KERNEL_TRACE") == "1"
    res = run_bass_kernel_spmd(_NC, in_maps, core_ids=list(range(N_CORES)),
                               trace=trace)
    last_exec_time_ns = res.exec_time_ns
    global last_results
    last_results = res.results
    out = np.zeros((SEQ, DIM), np.float32)
    for m in range(N_CORES):
        om = res.results[m]["out"]
        out[P * m:P * (m + 1)] = om[0:P]
        out[P * (m + 8):P * (m + 9)] = om[P:MQ]
    return out[None, ...].astype(np.float32)


# revision 37
# speedup vs baseline: 1.1024x; 1.0572x over previous
"""Trainium2 Bass kernel for MQA attention with RMSNorm + positional bias.

Reference computation (per core, seq-sharded over 8 cores):
  xn = rmsnorm(x) * gamma
  q = (xn @ wq) * scale   (16 heads x 128)     k = xn @ wk    v = xn @ wv
  sim = q @ k^T + pos_bias ; masked (non-causal entries := 1e-10)
  attn = softmax(sim); out = (attn @ v, concat heads) @ wo

Sharding: core m owns query rows of global 128-row tiles {m, m+8}. K/V
(shared MQA head) are computed from own rows and AllGathered in ONE
combined collective (k fp32 cols [0:256], v fp32 cols [256:512]).

Design (v2 rewrite of the 421us baseline):
- pos_bias AND the causal mask are folded into one additive host tensor
  pb (bf16, -1e9 at masked cols incl. a packed diag slot). pb is
  accumulated into the sim PSUM by the PE itself (identity-weight bf16
  matmul, 1 cycle/row) - no elementwise mask/mult pass and no gated-k
  copies. Masked cols then vanish through exp naturally, and the row max
  (V) can read raw PSUM.
- psim chunks are exactly 512 wide (1 PSUM bank): [512,384+128diag] for
  t=0 (7 main tiles), [512,512,512,384+128diag] for t=1 (15 main tiles).
- softmax denominator comes free via exp's accum_out; the masked-value
  correction of the reference (exp(1e-10-m) per masked col) is
  cnt*em as in the baseline.
- Normalization is folded into the P^T transposes: rhs = diag(rec)
  instead of identity, so P^T arrives pre-normalized.
- attn@v runs transposed per 4-head group: out^T[d, 4*128 q] with v
  tiles as weights - output lands directly in oT layout (no per-unit
  normalize chain, no 32 output transposes), and the masked-value
  output correction em*rec*stail rides one extra matmul with
  rhs = blockdiag(c_h), c_h = em_h*rec_h.
- k/v projections accumulate chunk-by-chunk interleaved with the x^T
  transposes so the combined AllGather triggers ~20us in; qproj +
  wq streaming fill the mesh-latency window.
"""

import os

import numpy as np
import ml_dtypes

import concourse.bass as bass
import concourse.mybir as mybir
import concourse.tile as tile
from concourse import bacc, masks
from concourse.bass_utils import run_bass_kernel_spmd

SEQ = 2048
DIM = 2048
H = 16
DH = 128
P = 128
N_CORES = 8
MQ = SEQ // N_CORES      # 256 query rows per core
NQT = MQ // P            # 2 query tiles per core
CD = DIM // P            # 16 contraction chunks
NS = SEQ // P            # 16 seq tiles
NMT = {0: 7, 1: 15}      # main key tiles per unit type
NCH = {0: 2, 1: 4}       # 512-wide sim chunks per unit type (diag packed)
LW = {0: 1024, 1: 2048}  # pb / pexp width per unit type
NT = H * NQT             # 32 pipeline units
GH = 4                   # heads per attn@v group
WOPF = 8                 # wo heads prefetched during the pipeline
SCALE = DH ** -0.5
EPS = 1e-5

FP = mybir.dt.float32
F16 = mybir.dt.float16
FR = mybir.dt.float32r
BF = mybir.dt.bfloat16
AF = mybir.ActivationFunctionType
ALU = mybir.AluOpType
AX = mybir.AxisListType

last_exec_time_ns = None


def _rms_scale_rows(nc, pool, xt, tag):
    """In-place x *= rsqrt(mean(x^2)+eps) for a [P, DIM] tile."""
    sq = pool.tile([P, DIM], FP, tag="sq_scratch", name="sq_scratch", bufs=1)
    ssq = pool.tile([P, 1], FP, tag=f"ssq{tag}", name=f"ssq{tag}")
    nc.scalar.activation(sq[:], xt[:], AF.Square, accum_out=ssq[:])
    nc.vector.tensor_scalar(ssq[:], ssq[:], 1.0 / DIM, EPS, ALU.mult, ALU.add)
    nc.scalar.sqrt(ssq[:], ssq[:])
    nc.vector.reciprocal(ssq[:], ssq[:])
    nc.vector.tensor_scalar_mul(xt[:], xt[:], ssq[:])


def build():
    nc = bacc.Bacc("TRN2", target_bir_lowering=False, debug=False,
                   num_devices=N_CORES)
    xq_d = nc.dram_tensor("xq", [NQT * 8 * P, 256], FP,
                          kind="ExternalInput")
    pbA_d = nc.dram_tensor("pbA", [H * P, LW[0]], BF, kind="ExternalInput")
    pbB_d = nc.dram_tensor("pbB", [H * P, LW[1]], BF, kind="ExternalInput")
    lo_d = nc.dram_tensor("lones", [P, P], BF, kind="ExternalInput")
    cnt_d = nc.dram_tensor("cnt", [P, NQT], FP, kind="ExternalInput")
    hz_d = nc.dram_tensor("hz", [P, NQT * NS], BF, kind="ExternalInput")
    wq_d = nc.dram_tensor("wq", [P, H * CD * DH], FR, kind="ExternalInput")
    wk_d = nc.dram_tensor("wk", [P, CD * DH], FR, kind="ExternalInput")
    wv_d = nc.dram_tensor("wv", [P, CD * DH], FR, kind="ExternalInput")
    wo_d = nc.dram_tensor("wo", [H * DH, DIM], BF, kind="ExternalInput")
    out_d = nc.dram_tensor("out", [MQ, DIM], FP, kind="ExternalOutput")
    DBG = os.environ.get("KERNEL_DEBUG") == "1"
    if DBG:
        dbg_qT = nc.dram_tensor("dbg_qT", [P, H * MQ], FP,
                                kind="ExternalOutput")
        dbg_kT = nc.dram_tensor("dbg_kT", [P, SEQ], FP, kind="ExternalOutput")
        dbg_vsb = nc.dram_tensor("dbg_vsb", [P, NS * P], FP,
                                 kind="ExternalOutput")
        dbg_st = nc.dram_tensor("dbg_st", [P, NQT * DH], FP,
                                kind="ExternalOutput")
        dbg_oT = nc.dram_tensor("dbg_oT", [P, H * MQ], FP,
                                kind="ExternalOutput")
        dbg_px = nc.dram_tensor("dbg_px", [P, LW[0] + LW[1]], FP,
                                kind="ExternalOutput")
        dbg_sc = nc.dram_tensor("dbg_sc", [P, 8], FP, kind="ExternalOutput")

    with tile.TileContext(nc) as tc, \
         tc.tile_pool(name="singles", bufs=1) as singles:
        # ---- persistent tiles --------------------------------------------
        ident = singles.tile([P, P], FP, tag="ident", name="ident")
        masks.make_identity(nc, ident[:])
        identb = singles.tile([P, P], BF, tag="identb", name="identb")
        masks.make_identity(nc, identb[:])
        cnt = singles.tile([P, NQT], FP, tag="cnt", name="cnt")

        qT = singles.tile([P, H, MQ], FR, tag="qT", name="qT")
        kTA = singles.tile([P, LW[0]], FR, tag="kTA", name="kTA")
        kTB = singles.tile([P, LW[1]], FR, tag="kTB", name="kTB")
        vsb = singles.tile([P, NS, P], BF, tag="vsb", name="vsb")
        vown = singles.tile([P, NQT, P], BF, tag="vown", name="vown")
        stail = singles.tile([P, NQT, DH], BF, tag="stail", name="stail")
        oT = singles.tile([P, H, MQ], BF, tag="oT", name="oT")
        wo_sb = singles.tile([P, WOPF, DIM], BF, tag="wo_sb", name="wo_sb")
        hz_sb = singles.tile([P, NQT, NS], BF, tag="hz", name="hz_sb")
        lones = singles.tile([P, P], BF, tag="lones", name="lones")
        ones1 = singles.tile([1, P], BF, tag="ones1", name="ones1")
        vsuf = singles.tile([1, DH], BF, tag="vsuf", name="vsuf")

        with tc.tile_pool(name="xnTqp", bufs=1) as xnTqp, \
             tc.tile_pool(name="pos", bufs=1) as posp, \
             tc.tile_pool(name="wqp", bufs=3) as wqp:
            xnT = xnTqp.tile([P, CD, MQ], FR, tag="xnT", name="xnT")
            WQH = {}
            PB = {}

            def qproj_dma(h, eng):
                wqh = wqp.tile([P, CD, DH], FR, tag="wqh",
                               name="wqh", bufs=3)
                for cb in range(4):
                    eng.dma_start(
                        out=wqh[:, cb * 4:(cb + 1) * 4, :],
                        in_=wq_d[:, h * CD * DH + cb * 512:
                                 h * CD * DH + (cb + 1) * 512])
                WQH[h] = wqh

            def pb_prefetch(i, eng0, eng1):
                h, t = i // NQT, i % NQT
                pbsel = pbA_d if t == 0 else pbB_d
                pb_t = posp.tile([P, LW[t]], BF, tag=f"pb{t}",
                                 name="pb", bufs=3)
                half = LW[t] // 2
                eng0.dma_start(out=pb_t[:, 0:half],
                               in_=pbsel[h * P:(h + 1) * P, 0:half])
                eng1.dma_start(out=pb_t[:, half:],
                               in_=pbsel[h * P:(h + 1) * P, half:])
                PB[i] = pb_t

            # ---- phase 0+1: xn^T, k/v proj, combined AllGather -----------
            with tc.tile_pool(name="ph0", bufs=1) as ph0, \
                 tc.tile_pool(name="kvw", bufs=1) as kvwp, \
                 tc.tile_pool(name="dram", bufs=1, space="DRAM") as dramp, \
                 tc.tile_pool(name="pstr0", bufs=2, space="PSUM") as pstr0, \
                 tc.tile_pool(name="psk", bufs=1, space="PSUM") as psk, \
                 tc.tile_pool(name="psv", bufs=1, space="PSUM") as psv, \
                 tc.tile_pool(name="pstv", bufs=1, space="PSUM") as pstv:
                wk_sb = kvwp.tile([P, CD, DH], FR, tag="wk", name="wk_sb")
                wv_sb = kvwp.tile([P, CD, DH], FR, tag="wv", name="wv_sb")
                # x load first: 8 column chunks spread over issue queues
                xnq = []
                engs = [nc.sync, nc.scalar, nc.gpsimd]
                ei = 0
                for t in range(NQT):
                    xt = ph0.tile([P, DIM], FP, tag=f"xq{t}", name=f"xq{t}")
                    for ci in range(8):
                        blk = (t * 8 + ci) * P
                        engs[ei % 3].dma_start(
                            out=xt[:, ci * 256:(ci + 1) * 256],
                            in_=xq_d[blk:blk + P, :])
                        ei += 1
                    xnq.append(xt)
                for ci in range(4):
                    nc.scalar.dma_start(
                        out=wk_sb[:, ci * 4:(ci + 1) * 4, :],
                        in_=wk_d[:, ci * 512:(ci + 1) * 512])
                    nc.gpsimd.dma_start(
                        out=wv_sb[:, ci * 4:(ci + 1) * 4, :],
                        in_=wv_d[:, ci * 512:(ci + 1) * 512])
                nc.vector.memset(ones1[:], 1.0)
                nc.scalar.dma_start(out=lones[:], in_=lo_d[:])
                nc.scalar.dma_start(out=hz_sb[:], in_=hz_d[:])
                nc.sync.dma_start(out=cnt[:], in_=cnt_d[:])
                for t in range(NQT):
                    _rms_scale_rows(nc, ph0, xnq[t], f"q{t}")
                # transposes interleaved with k/v projection accumulation
                # (gamma is folded into wq/wk/wv host-side)
                pk = psk.tile([P, MQ], FP, tag="pk", name="pk")
                pv = psv.tile([P, MQ], FP, tag="pv", name="pv")
                # t0 transposes stream while t1's x still loads; k/v proj
                # accumulate full-width (fp32r needs free>=256 for 1c/row)
                for c in range(CD):
                    pt = pstr0.tile([P, P], FP, tag="trq", name="trq")
                    nc.tensor.transpose(pt[:], xnq[0][:, c * P:(c + 1) * P],
                                        ident[:])
                    if c % 2 == 0:
                        nc.vector.tensor_copy(xnT[:, c, 0:P], pt[:])
                    else:
                        nc.scalar.copy(xnT[:, c, 0:P], pt[:])
                for c in range(CD + 1):
                    if c < CD:
                        pt = pstr0.tile([P, P], FP, tag="trq", name="trq")
                        nc.tensor.transpose(
                            pt[:], xnq[1][:, c * P:(c + 1) * P], ident[:])
                        if c % 2 == 0:
                            nc.vector.tensor_copy(xnT[:, c, P:MQ], pt[:])
                        else:
                            nc.scalar.copy(xnT[:, c, P:MQ], pt[:])
                    if c > 0:
                        nc.tensor.matmul(pk[:], lhsT=wk_sb[:, c - 1, :],
                                         rhs=xnT[:, c - 1, :],
                                         start=(c == 1), stop=(c == CD))
                        nc.tensor.matmul(pv[:], lhsT=wv_sb[:, c - 1, :],
                                         rhs=xnT[:, c - 1, :],
                                         start=(c == 1), stop=(c == CD))
                # k bounce (fp32) and v bounce (bf16, = vown layout)
                kbsb = kvwp.tile([P, MQ], F16, tag="kbsb", name="kbsb")
                nc.scalar.copy(kbsb[:], pk[:])
                vTs = kvwp.tile([P, MQ], FP, tag="vTs", name="vTs")
                nc.vector.tensor_copy(vTs[:], pv[:])
                for t in range(NQT):
                    ptv = pstv.tile([P, P], FP, tag="vtr", name="vtr")
                    nc.tensor.transpose(ptv[:], vTs[:, t * P:(t + 1) * P],
                                        ident[:])
                    nc.vector.tensor_copy(vown[:, t, :], ptv[:])
                # fused k(fp16-bits)+v(bf16) bounce: ONE mesh
                kv_bounce = dramp.tile([P, 2 * MQ], BF, tag="kvb",
                                       name="kv_bounce")
                kv_ag = dramp.tile([N_CORES, P, 2 * MQ], BF, tag="kvag",
                                   name="kv_ag", addr_space="Shared")
                nc.scalar.dma_start(out=kv_bounce[:, 0:MQ],
                                    in_=kbsb[:].bitcast(BF))
                nc.scalar.dma_start(out=kv_bounce[:, MQ:2 * MQ],
                                    in_=vown[:, :, :])
                rg = [list(range(N_CORES))]
                nc.gpsimd.collective_compute(
                    "AllGather", ALU.bypass, replica_groups=rg,
                    ins=[kv_bounce[:].opt()], outs=[kv_ag[:, :, :].opt()])
                # k unpack into per-t layouts (wide permuted-AP DMAs on the
                # gpsimd queue, which is dead between the two collectives)
                kstgA = kvwp.tile([P, LW[0]], F16, tag="kstgA",
                                  name="kstgA")
                kstgB = kvwp.tile([P, LW[1]], F16, tag="kstgB",
                                  name="kstgB")
                nc.scalar.copy(kstgA[:, NMT[0] * P:], pk[:, 0:P])
                nc.scalar.copy(kstgB[:, NMT[1] * P:], pk[:, P:MQ])

                def unpack(dst, r0, r1, h0, cast=None):
                    in_ = kv_ag[r0:r1, :, h0 * P:(h0 + 1) * P] \
                        .transpose([1, 0, 2])
                    if cast is not None:
                        in_ = in_.bitcast(cast)
                    nc.gpsimd.dma_start(out=dst, in_=in_)
                unpack(kstgA[:, 0:3 * P], 0, 3, 0, F16)
                unpack(kstgA[:, 3 * P:5 * P], 3, 5, 0, F16)
                unpack(kstgA[:, 5 * P:7 * P], 5, 7, 0, F16)
                nc.vector.tensor_copy(kTA[:], kstgA[:])
                unpack(kstgB[:, 0:4 * P], 0, 4, 0, F16)
                unpack(kstgB[:, 4 * P:8 * P], 4, 8, 0, F16)
                nc.vector.tensor_copy(kTB[:, 0:8 * P], kstgB[:, 0:8 * P])
                unpack(kstgB[:, 8 * P:12 * P], 0, 4, 1, F16)
                unpack(kstgB[:, 12 * P:15 * P], 4, 7, 1, F16)
                nc.vector.tensor_copy(kTB[:, 8 * P:], kstgB[:, 8 * P:])
                # prologue prefetches at the tail of phase-1's scalar queue:
                # transfers start only once the kproj path stops needing HBM
                for h in range(12):
                    qproj_dma(h, nc.scalar)
                for i in range(6):
                    pb_prefetch(i, nc.scalar, nc.scalar)
                VCTX = unpack

            # ---- phase 2: q proj + attention pipeline --------------------
            with tc.tile_pool(name="pexpp", bufs=1) as pexpp, \
                 tc.tile_pool(name="pts", bufs=1) as ptsp, \
                 tc.tile_pool(name="st", bufs=1) as stp, \
                 tc.tile_pool(name="ps2", bufs=1, space="PSUM") as ps2:

                def qproj_mm(h):
                    wqh = WQH.pop(h)
                    pq = ps2.tile([P, MQ], FP, tag="psim", name="pq",
                                  bufs=5)
                    for c in range(CD):
                        nc.tensor.matmul(pq[:], lhsT=wqh[:, c, :],
                                         rhs=xnT[:, c, :],
                                         start=(c == 0), stop=(c == CD - 1))
                    nc.scalar.copy(qT[:, h, :], pq[:])

                S = {}    # per-unit pipeline state
                PTD = {}  # per-(group, t) P^T staging tiles
                DC = {}   # per-(group, t) blockdiag(em*rec) tiles

                def stage_mm(i):
                    h, t = i // NQT, i % NQT
                    nch = NCH[t]
                    pb_t = PB.pop(i)
                    kTt = kTA if t == 0 else kTB
                    qsl = qT[:, h, t * P:(t + 1) * P]
                    psims = []
                    for c in range(nch):
                        psim = ps2.tile([P, 512], FP, tag="psim",
                                        name="psim", bufs=5)
                        nc.tensor.matmul(psim[:], lhsT=qsl,
                                         rhs=kTt[:, c * 512:(c + 1) * 512],
                                         start=True, stop=False)
                        nc.tensor.matmul(psim[:], lhsT=identb[:],
                                         rhs=pb_t[:, c * 512:(c + 1) * 512],
                                         start=False, stop=True)
                        psims.append(psim)
                    mxc = stp.tile([P, 4], FP, tag="mxc", name="mxc",
                                   bufs=3)
                    for c in range(nch):
                        nc.vector.tensor_reduce(mxc[:, c:c + 1], psims[c][:],
                                                axis=AX.X, op=ALU.max)
                    negmax = stp.tile([P, 1], FP, tag="negmax",
                                      name="negmax", bufs=4)
                    nc.vector.tensor_reduce(negmax[:], mxc[:, 0:nch],
                                            axis=AX.X, op=ALU.max,
                                            negate=True)
                    nc.vector.tensor_scalar(negmax[:], negmax[:], 1.0, 0.0,
                                            ALU.mult, ALU.min)
                    S[i] = {"psims": psims, "negmax": negmax}

                def stage_exp(i):
                    h, t = i // NQT, i % NQT
                    g, hh = h // GH, h % GH
                    nch = NCH[t]
                    st = S[i]
                    negmax = st["negmax"]
                    pexp = pexpp.tile([P, LW[t]], BF, tag=f"pexp{t}",
                                      name="pexp", bufs=3)
                    sexp = stp.tile([P, 4], FP, tag="sexp", name="sexp",
                                    bufs=3)
                    for c in range(nch):
                        nc.scalar.activation(pexp[:, c * 512:(c + 1) * 512],
                                             st["psims"][c][:], AF.Exp,
                                             bias=negmax[:],
                                             accum_out=sexp[:, c:c + 1])
                    em = stp.tile([P, 1], FP, tag="em", name="em", bufs=4)
                    nc.scalar.activation(em[:], negmax[:], AF.Exp)
                    ssum = stp.tile([P, 1], FP, tag="ssum", name="ssum",
                                    bufs=4)
                    nc.vector.tensor_reduce(ssum[:], sexp[:, 0:nch],
                                            axis=AX.X, op=ALU.add)
                    ve = nc.vector
                    ssum2 = stp.tile([P, 1], FP, tag="ssum2", name="ssum2",
                                     bufs=4)
                    nc.vector.scalar_tensor_tensor(
                        ssum2[:], cnt[:, t:t + 1], em[:], ssum[:],
                        op0=ALU.mult, op1=ALU.add)
                    rec = stp.tile([P, 1], FP, tag="rec", name="rec", bufs=4)
                    nc.vector.reciprocal(rec[:], ssum2[:])
                    drec = stp.tile([P, P], BF, tag="drec", name="drec",
                                    bufs=4)
                    ve.tensor_scalar_mul(drec[:], identb[:], rec[:])
                    cc_ = stp.tile([P, 1], FP, tag="cc", name="cc", bufs=4)
                    ve.tensor_tensor(cc_[:], em[:], rec[:], op=ALU.mult)
                    if (g, t) not in DC:
                        DC[(g, t)] = stp.tile([P, GH * P], BF, tag=f"dc{t}",
                                              name="dc", bufs=2)
                    ve.tensor_scalar_mul(
                        DC[(g, t)][:, hh * P:(hh + 1) * P], identb[:], cc_[:])
                    st["pexp"] = pexp
                    st["drec"] = drec
                    if DBG and i < 2:
                        pxs = stp.tile([P, LW[1]], FP, tag="pxs",
                                       name="pxs", bufs=2)
                        nc.scalar.copy(pxs[:, 0:LW[t]], pexp[:])
                        off = 0 if i == 0 else LW[0]
                        nc.sync.dma_start(out=dbg_px[:, off:off + LW[t]],
                                          in_=pxs[:, 0:LW[t]])
                        scs = stp.tile([P, 4], FP, tag="scs", name="scs",
                                       bufs=2)
                        nc.vector.tensor_copy(scs[:, 0:1], negmax[:])
                        nc.vector.tensor_copy(scs[:, 1:2], em[:])
                        nc.vector.tensor_copy(scs[:, 2:3], ssum2[:])
                        nc.vector.tensor_copy(scs[:, 3:4], rec[:])
                        nc.sync.dma_start(out=dbg_sc[:, 4 * i:4 * i + 4],
                                          in_=scs[:])

                def stage_pt(i):
                    h, t = i // NQT, i % NQT
                    g, hh = h // GH, h % GH
                    st = S.pop(i)
                    pexp, drec = st["pexp"], st["drec"]
                    nmt1 = NMT[t] + 1
                    if (g, t) not in PTD:
                        PTD[(g, t)] = ptsp.tile([P, nmt1, GH * P], BF,
                                                tag=f"PT{t}", name="PT",
                                                bufs=2)
                    PT = PTD[(g, t)]
                    for g0 in range(0, nmt1, 4):
                        ppt = ps2.tile([P, 4 * P], FP, tag="ppt",
                                       name="ppt", bufs=2)
                        for s4 in range(4):
                            # normal-mode matmul (PE transpose mode would
                            # ignore rhs): out[k,q] = pexp[q,k] * rec[q]
                            nc.tensor.matmul(
                                ppt[:, s4 * P:(s4 + 1) * P],
                                lhsT=pexp[:, (g0 + s4) * P:(g0 + s4 + 1) * P],
                                rhs=drec[:], start=True, stop=True)
                        dst = PT[:, g0:g0 + 4, hh * P:(hh + 1) * P]
                        if (g0 // 4) % 2 == 0:
                            nc.vector.tensor_copy(dst, ppt[:])
                        else:
                            nc.scalar.copy(dst, ppt[:])

                PO = {}   # per-(group, t) out^T PSUM accumulator

                def attnv_slice(g, sl):
                    # slices 0-2: t=0 (s 0-3, s 4-7, corr+drain)
                    # slices 3-7: t=1 (s 0-3, 4-7, 8-11, 12-15, corr+drain)
                    t = 0 if sl < 3 else 1
                    nmt = NMT[t]
                    base = sl if t == 0 else sl - 3
                    is_mm = (t == 0 and sl < 2) or (t == 1 and sl < 7)
                    if is_mm and base == 0:
                        PO[(g, t)] = ps2.tile([P, GH * P], FP, tag="poT",
                                              name="poT", bufs=1)
                    po = PO[(g, t)]
                    if is_mm:
                        PT = PTD[(g, t)]
                        for s in range(base * 4, base * 4 + 4):
                            lhsT = vown[:, t, :] if s == nmt \
                                else vsb[:, s, :]
                            nc.tensor.matmul(po[:], lhsT=lhsT,
                                             rhs=PT[:, s, :],
                                             start=(s == 0), stop=False)
                    else:
                        nc.tensor.matmul(po[:], lhsT=stail[:, t, :],
                                         rhs=DC.pop((g, t))[:],
                                         start=False, stop=True)
                        del PTD[(g, t)]
                        del PO[(g, t)]
                        dst = oT[:, g * GH:(g + 1) * GH,
                                 t * P:(t + 1) * P]
                        if t == 0:
                            nc.scalar.copy(dst, po[:])
                        else:
                            nc.vector.tensor_copy(dst, po[:])

                # prologue: qproj h0-11 mm's fill the k-mesh window
                for h in range(12):
                    qproj_mm(h)
                unpack = VCTX
                STEPS = NT + 2
                for i in range(STEPS):
                    if i == 1:
                        unpack(vsb[:, 0:4, :], 0, 4, 2)
                        unpack(vsb[:, 4:8, :], 4, 8, 2)
                        unpack(vsb[:, 8:12, :], 0, 4, 3)
                        unpack(vsb[:, 12:16, :], 4, 8, 3)
                    if i == 4:
                        # stail[t] = sum_{j > global row} v_j (bf16);
                        # borrows ps2 "ppt" psum slots
                        for t_ in range(NQT):
                            psv1 = ps2.tile([1, 4 * P], FP, tag="ppt",
                                            name="psv1", bufs=2)
                            for s in range(NS):
                                nc.tensor.matmul(
                                    psv1[:, 0:DH],
                                    lhsT=hz_sb[:, t_, s:s + 1],
                                    rhs=vsb[:, s, :],
                                    start=(s == 0), stop=(s == NS - 1))
                            nc.scalar.copy(vsuf[:], psv1[:, 0:DH])
                            pst = ps2.tile([P, 4 * P], FP, tag="ppt",
                                           name="pst", bufs=2)
                            nc.tensor.matmul(pst[:, 0:DH], lhsT=lones[:],
                                             rhs=vown[:, t_, :],
                                             start=True, stop=False)
                            nc.tensor.matmul(pst[:, 0:DH], lhsT=ones1[:],
                                             rhs=vsuf[:],
                                             start=False, stop=True,
                                             skip_group_check=True)
                            nc.vector.tensor_copy(stail[:, t_, :],
                                                  pst[:, 0:DH])
                    if i + 6 < NT:
                        pb_prefetch(i + 6, nc.gpsimd, nc.gpsimd)
                    if i < NT:
                        h, t = i // NQT, i % NQT
                        if t == 0:
                            if h + 12 < H:
                                qproj_dma(h + 12, nc.scalar if (h % 2 == 0)
                                          else nc.gpsimd)
                            if 12 <= h + 11 < H:
                                qproj_mm(h + 11)
                        stage_mm(i)
                        stage_exp(i)
                    if 0 <= i - 2 < NT:
                        stage_pt(i - 2)
                    j = i - 10
                    if j >= 0 and j // 8 < NT // 8:
                        attnv_slice(j // 8, j % 8)
                    if i < NT and i % 4 == 1 and i // 4 < WOPF:
                        hw_ = i // 4
                        nc.gpsimd.dma_start(
                            out=wo_sb[:, hw_, :],
                            in_=wo_d[hw_ * DH:(hw_ + 1) * DH, :])
                for j in range(STEPS - 10, NT):
                    attnv_slice(j // 8, j % 8)

        if DBG:
            with tc.tile_pool(name="dbgp", bufs=2) as dbgp:
                def dump(dst, src_ap, width):
                    stg = dbgp.tile([P, width], FP, tag="dstg",
                                    name="dstg", bufs=2)
                    nc.scalar.copy(stg[:], src_ap)
                    nc.sync.dma_start(out=dst[:, 0:width], in_=stg[:])
                dump(dbg_qT, qT[:, :, :], H * MQ)
                dump(dbg_kT, kTB[:], LW[1])
                dump(dbg_vsb, vsb[:, :, :], NS * P)
                dump(dbg_st, stail[:, :, :], NQT * DH)
                dump(dbg_oT, oT[:, :, :], H * MQ)

        # ---- phase 4: output projection (bf16, JIT wo streaming) ---------
        with tc.tile_pool(name="osb", bufs=2) as osbp, \
             tc.tile_pool(name="wof", bufs=1) as wofp, \
             tc.tile_pool(name="ps_out", bufs=NQT * (DIM // 512),
                          space="PSUM") as ps_out:
            pouts = []
            for t in range(NQT):
                for nk in range(DIM // 512):
                    pouts.append(ps_out.tile([P, 512], FP, tag="pout",
                                             name=f"pout{t}_{nk}"))
            # stream the non-prefetched wo heads all at once, split across
            # issue queues, so the h>=WOPF matmuls are never DMA-paced
            wfs = {}
            weng = [nc.gpsimd, nc.scalar, nc.sync]
            for h in range(WOPF, H):
                wo_b = wofp.tile([P, DIM], BF, tag=f"wof{h}", name="wo_b")
                for q2 in range(2):
                    weng[(h + q2) % 3].dma_start(
                        out=wo_b[:, q2 * 1024:(q2 + 1) * 1024],
                        in_=wo_d[h * DH:(h + 1) * DH,
                                 q2 * 1024:(q2 + 1) * 1024])
                wfs[h] = wo_b
            # chunk 7 aliases the PSUM bank the pipeline's poT holds until
            # the flush's last drain - defer it so the h-major stream never
            # head-of-line blocks on that bank; drain each chunk right after
            # its last accumulation so stores overlap remaining compute
            nwk = DIM // 512
            osbs = {}
            for t in range(NQT):
                osbs[t] = osbp.tile([P, DIM], FP, tag="osb", name="osb")

            def pout_mm(c2, h):
                t, nk = c2 // nwk, c2 % nwk
                wo_h = wo_sb[:, h, :] if h < WOPF else wfs[h][:]
                nc.tensor.matmul(pouts[c2][:],
                                 lhsT=oT[:, h, t * P:(t + 1) * P],
                                 rhs=wo_h[:, nk * 512:(nk + 1) * 512],
                                 start=(h == 0), stop=(h == H - 1))

            def pout_drain(c2):
                t, nk = c2 // nwk, c2 % nwk
                eng = nc.scalar if c2 % 2 == 0 else nc.vector
                if c2 % 2 == 0:
                    nc.scalar.copy(osbs[t][:, nk * 512:(nk + 1) * 512],
                                   pouts[c2][:])
                else:
                    nc.vector.tensor_copy(
                        osbs[t][:, nk * 512:(nk + 1) * 512], pouts[c2][:])
                nc.sync.dma_start(
                    out=out_d[t * P:(t + 1) * P, nk * 512:(nk + 1) * 512],
                    in_=osbs[t][:, nk * 512:(nk + 1) * 512])
            for h in range(H):
                for c2 in range(7):
                    pout_mm(c2, h)
                if h == H - 1:
                    for c2 in range(7):
                        pout_drain(c2)
            for h in range(H):
                pout_mm(7, h)
            pout_drain(7)

    nc.compile()
    return nc


_NC = None


def kernel(**inputs):
    global _NC, last_exec_time_ns
    x = np.asarray(inputs["x"], dtype=np.float32)[0]          # [SEQ, DIM]
    pos = np.asarray(inputs["pos_bias"], dtype=np.float32)    # [H, SEQ, SEQ]
    gamma = np.asarray(inputs["gamma"], dtype=np.float32)
    wq = np.ascontiguousarray(np.asarray(inputs["wq"], dtype=np.float32))
    wk = np.ascontiguousarray(np.asarray(inputs["wk"], dtype=np.float32))
    wv = np.ascontiguousarray(np.asarray(inputs["wv"], dtype=np.float32))
    wo = np.ascontiguousarray(np.asarray(inputs["wo"], dtype=np.float32))

    if _NC is None:
        _NC = build()

    x = np.ascontiguousarray(x)
    # device layouts: wq [P, h, c, d], wk/wv [P, c, d]; gamma folded in
    g_ = gamma[:, None]
    wqs = np.ascontiguousarray(
        (wq * g_ * np.float32(SCALE)).reshape(CD, P, H, DH)
        .transpose(1, 2, 0, 3).reshape(P, H * CD * DH))
    wk_r = np.ascontiguousarray(
        (wk * g_).reshape(CD, P, DH).transpose(1, 0, 2).reshape(P, CD * DH))
    wv_r = np.ascontiguousarray(
        (wv * g_).reshape(CD, P, DH).transpose(1, 0, 2).reshape(P, CD * DH))
    wo_b = np.ascontiguousarray(wo.astype(ml_dtypes.bfloat16))
    NEG = np.float32(-1e9)
    jidx = np.arange(SEQ)
    jl = np.arange(P)
    pidx = np.arange(P)
    triu_strict = (jl[None, :] > pidx[:, None])               # [row, col]
    lones = np.ascontiguousarray(triu_strict.T.astype(ml_dtypes.bfloat16))
    in_maps = []
    for m in range(N_CORES):
        pbs = {}
        for t in range(NQT):
            g = m + 8 * t
            gq = slice(P * g, P * (g + 1))
            nm = NMT[t]
            pb = np.empty((H, P, LW[t]), np.float32)
            # main region: global cols [0 : nm*P], visible iff j < P*g
            pb[:, :, :nm * P] = np.where(
                (jidx[:nm * P] < P * g)[None, None, :],
                pos[:, gq, :nm * P], NEG)
            # diag slot: own tile with strict-upper mask
            pb[:, :, nm * P:] = np.where(
                triu_strict[None, :, :], NEG,
                pos[:, gq, P * g:P * (g + 1)])
            pbs[t] = np.ascontiguousarray(
                pb.reshape(H * P, LW[t]).astype(ml_dtypes.bfloat16))
        hz = np.zeros((P, NQT, NS), np.float32)
        cnt_m = np.zeros((P, NQT), np.float32)
        xq_m = np.zeros((MQ, DIM), np.float32)
        for t in range(NQT):
            g = m + 8 * t
            xq_m[t * P:(t + 1) * P] = x[P * g:P * (g + 1)]
            for s in range(NS):
                hz[:, t, s] = ((s * P + pidx) >= P * (g + 1)).astype(
                    np.float32)
            cnt_m[:, t] = (SEQ - 1) - (P * g + pidx)
        xq2 = (xq_m.reshape(NQT, P, 8, 256).transpose(0, 2, 1, 3)
               .reshape(NQT * 8 * P, 256))
        in_maps.append({
            "xq": np.ascontiguousarray(xq2),
            "pbA": pbs[0], "pbB": pbs[1],
            "lones": lones,
            "cnt": np.ascontiguousarray(cnt_m),
            "hz": np.ascontiguousarray(
                hz.reshape(P, NQT * NS).astype(ml_dtypes.bfloat16)),
            "wq": wqs, "wk": wk_r, "wv": wv_r, "wo": wo_b,
        })
    trace = os.environ.get("KERNEL_TRACE") == "1"
    res = run_bass_kernel_spmd(_NC, in_maps, core_ids=list(range(N_CORES)),
                               trace=trace)
    last_exec_time_ns = res.exec_time_ns
    global last_results
    last_results = res.results
    out = np.zeros((SEQ, DIM), np.float32)
    for m in range(N_CORES):
        om = res.results[m]["out"]
        out[P * m:P * (m + 1)] = om[0:P]
        out[P * (m + 8):P * (m + 9)] = om[P:MQ]
    return out[None, ...].astype(np.float32)
